# revision 16
# baseline (speedup 1.0000x reference)
"""Trainium2 Bass kernel for nn_MimiAttention (sliding-window causal attention).

Reference math (T=4096, HID=1024, 16 heads x 64 dims, window 512, RoPE):
  q = rope(x @ wq.T); k = rope(x @ wk.T); v = x @ wv.T
  ctx = sdpa(q, k, v, causal, local_window=(512, 0), scale=1/8)
  out = ctx @ wo.T

Sharding: sequence-parallel across 8 NeuronCores, zero communication.
Core c owns queries [c*512, (c+1)*512) and recomputes k/v over its kv
window [c*512-512, (c+1)*512) (halo recompute).

On-device layout: everything transposed (feature dim on partitions).
Softmax without max-subtraction (scores are small: |S/8| < ~4), row sums
via a ones-column appended to V, triangle masks as bf16 multiplies on
exp(S^T), all 16 head denominators inverted in one batched reciprocal.
RoPE in the transposed layout: partner-dim swap via 4 small SBUF->SBUF
DMAs (partition shift), then two table multiplies + add.
"""

import sys

sys.path.insert(0, "/opt/trn_rl_repo")

import numpy as np
import ml_dtypes

T, HID, NH, HD = 4096, 1024, 16, 64
WINDOW = 512
ROPE_THETA = 10000.0
NCORES = 8
QR = T // NCORES          # 512 queries per core
KV = QR + WINDOW          # 1024 kv rows per core (incl. halo)
NB = KV // 128            # 8 kv blocks
QT = QR // 128            # 4 query tiles
HP = NH // 2              # 8 head pairs
FC = HID // 128           # 8 feature chunks

_CACHE = {}


def _build_program():
    import concourse.mybir as mybir
    import concourse.tile as tile
    from concourse import bacc

    f32 = mybir.dt.float32
    bf16 = mybir.dt.bfloat16
    Exp = mybir.ActivationFunctionType.Exp

    nc = bacc.Bacc("TRN2", target_bir_lowering=False, debug=False,
                   num_devices=NCORES)

    xT_d = nc.declare_dram_parameter("xT", [HID, KV], bf16, isOutput=False)
    wqT_d = nc.declare_dram_parameter("wqT", [HID, HID], bf16, isOutput=False)
    wkT_d = nc.declare_dram_parameter("wkT", [HID, HID], bf16, isOutput=False)
    wvT_d = nc.declare_dram_parameter("wvT", [HID, HID], bf16, isOutput=False)
    woT_d = nc.declare_dram_parameter("woT", [HID, HID], bf16, isOutput=False)
    vones_d = nc.declare_dram_parameter("vones", [KV, 16], bf16, isOutput=False)
    mlo_d = nc.declare_dram_parameter("mlo", [128, 128], bf16, isOutput=False)
    mhi_d = nc.declare_dram_parameter("mhi", [128, 128], bf16, isOutput=False)
    rc_d = nc.declare_dram_parameter("ropecos", [128, KV], bf16, isOutput=False)
    rs_d = nc.declare_dram_parameter("ropesin", [128, KV], bf16, isOutput=False)
    out_d = nc.declare_dram_parameter("out", [QR, HID], f32, isOutput=True)

    with tile.TileContext(nc) as tc:
        with (
            tc.tile_pool(name="const", bufs=1) as cpool,
            tc.tile_pool(name="psA", bufs=3, space="PSUM") as psA,
            tc.tile_pool(name="psS", bufs=3, space="PSUM") as psS,
            tc.tile_pool(name="psC", bufs=2, space="PSUM") as psC,
            tc.tile_pool(name="pP", bufs=4) as pP,
            tc.tile_pool(name="pR", bufs=3) as pR,
            tc.tile_pool(name="pW", bufs=2) as pW,
        ):
            # ---- constants / weights into SBUF (xt/wv first for v-proj) ----
            xt, wv_t = [], []
            for f in range(FC):
                t_ = cpool.tile([128, KV], bf16, tag=f"xt{f}", name=f"xt{f}")
                nc.sync.dma_start(t_[:], xT_d[f * 128:(f + 1) * 128, :])
                xt.append(t_)
                t_ = cpool.tile([128, HID], bf16, tag=f"wv{f}", name=f"wv{f}")
                nc.sync.dma_start(t_[:], wvT_d[f * 128:(f + 1) * 128, :])
                wv_t.append(t_)

            def load_rows(dram, n_free, tagp):
                ts_ = []
                for f in range(FC):
                    t_ = cpool.tile([128, n_free], bf16, tag=f"{tagp}{f}",
                                    name=f"{tagp}{f}")
                    nc.sync.dma_start(t_[:], dram[f * 128:(f + 1) * 128, :])
                    ts_.append(t_)
                return ts_

            wq_t = load_rows(wqT_d, HID, "wq")
            rc = cpool.tile([128, KV], bf16, tag="rc", name="rc")
            nc.sync.dma_start(rc[:], rc_d[:])
            rs = cpool.tile([128, KV], bf16, tag="rs", name="rs")
            nc.sync.dma_start(rs[:], rs_d[:])
            wk_t = load_rows(wkT_d, HID, "wk")
            mlo = cpool.tile([128, 128], bf16, tag="mlo", name="mlo")
            nc.sync.dma_start(mlo[:], mlo_d[:])
            mhi = cpool.tile([128, 128], bf16, tag="mhi", name="mhi")
            nc.sync.dma_start(mhi[:], mhi_d[:])
            wo_t = load_rows(woT_d, HID, "wo")

            qT = [cpool.tile([128, QR], bf16, tag=f"qT{h}", name=f"qT{h}")
                  for h in range(HP)]
            kT = [cpool.tile([128, KV], bf16, tag=f"kT{h}", name=f"kT{h}")
                  for h in range(HP)]
            vv = [cpool.tile([128, 16, 65], bf16, tag=f"vv{b}", name=f"vv{b}")
                  for b in range(NB)]
            ctx = [cpool.tile([128, QR], bf16, tag=f"ctx{h}", name=f"ctx{h}")
                   for h in range(HP)]
            sumsA = cpool.tile([14, QR], f32, tag="sumsA", name="sumsA")
            sumsB = cpool.tile([2, QR], f32, tag="sumsB", name="sumsB")

            # ---- v projection (+ ones column) ----
            for rb in range(NB):
                nc.sync.dma_start(vv[rb][:, :, 64:65],
                                  vones_d[rb * 128:(rb + 1) * 128, :])
                for d2 in range(2):
                    v_ps = psA.tile([128, 8, 64], f32, tag="pj", name="vps")
                    for f in range(FC):
                        nc.tensor.matmul(
                            v_ps[:], xt[f][:, rb * 128:(rb + 1) * 128],
                            wv_t[f][:, d2 * 512:(d2 + 1) * 512],
                            start=(f == 0), stop=(f == FC - 1))
                    nc.vector.tensor_copy(vv[rb][:, d2 * 8:(d2 + 1) * 8, 0:64],
                                          v_ps[:])

            # ---- RoPE: dst[:, dc0:dc0+512] = rope(src_ps) ----
            def rope_apply(src_ps, dst, tc0, dc0):
                n = 512
                raw = pR.tile([128, n], bf16, tag="rraw", name="rraw")
                nc.vector.tensor_copy(raw[:], src_ps[:])
                swp = pR.tile([128, n], bf16, tag="rswp", name="rswp")
                for g in range(4):
                    pg = (g ^ 1) * 32
                    nc.sync.dma_start(swp[g * 32:(g + 1) * 32, :],
                                      raw[pg:pg + 32, :])
                nc.vector.tensor_mul(dst[:, dc0:dc0 + n], raw[:],
                                     rc[:, tc0:tc0 + n])
                t2 = pR.tile([128, n], bf16, tag="rt2", name="rt2")
                nc.vector.tensor_mul(t2[:], swp[:], rs[:, tc0:tc0 + n])
                nc.vector.tensor_add(dst[:, dc0:dc0 + n],
                                     dst[:, dc0:dc0 + n], t2[:])

            # ---- q^T / k^T projections with RoPE ----
            def proj(hp):
                q_ps = psA.tile([128, QR], f32, tag="pj", name="qps")
                for f in range(FC):
                    nc.tensor.matmul(
                        q_ps[:], wq_t[f][:, hp * 128:(hp + 1) * 128],
                        xt[f][:, WINDOW:KV],
                        start=(f == 0), stop=(f == FC - 1))
                rope_apply(q_ps, qT[hp], WINDOW, 0)
                for rh in range(2):
                    k_ps = psA.tile([128, 512], f32, tag="pj", name="kps")
                    for f in range(FC):
                        nc.tensor.matmul(
                            k_ps[:], wk_t[f][:, hp * 128:(hp + 1) * 128],
                            xt[f][:, rh * 512:(rh + 1) * 512],
                            start=(f == 0), stop=(f == FC - 1))
                    rope_apply(k_ps, kT[hp], rh * 512, rh * 512)

            # ---- attention for one head pair (h0/h1 share st/p tiles) ----
            B_ORDER = [4, 5, 6, 7, 0, 1, 2, 3]  # b=4 first: full-width write
            LAG = 2

            def attn(hp):
                ctx_ps = [psC.tile([65, QR], f32, tag="ctx", name="ctxps")
                          for _ in range(2)]
                pbuf = {}

                def stage_st(b):
                    tlo, thi = max(0, b - 4), min(QT - 1, b)
                    ncols = (thi - tlo + 1) * 128
                    p = pP.tile([128, 2, 512], bf16, tag="p", name="p")
                    sts = [psS.tile([128, 512], f32, tag="st", name="st")
                           for _ in range(2)]
                    with tc.tile_critical():
                        for h01 in range(2):
                            po = h01 * 64
                            nc.tensor.matmul(
                                sts[h01][:, :ncols],
                                kT[hp][po:po + 64, b * 128:(b + 1) * 128],
                                qT[hp][po:po + 64, tlo * 128:(thi + 1) * 128],
                                start=True, stop=True, tile_position=(po, 0))
                    for h01 in range(2):
                        nc.scalar.activation(p[:, h01, :ncols],
                                             sts[h01][:, :ncols], Exp)
                    if b <= QT - 1:
                        c0 = (b - tlo) * 128
                        for h01 in range(2):
                            nc.vector.tensor_mul(p[:, h01, c0:c0 + 128],
                                                 p[:, h01, c0:c0 + 128], mlo[:])
                    if b >= 4:
                        for h01 in range(2):
                            nc.vector.tensor_mul(p[:, h01, 0:128],
                                                 p[:, h01, 0:128], mhi[:])
                    pbuf[b] = p

                def stage_pv(b):
                    tlo, thi = max(0, b - 4), min(QT - 1, b)
                    ncols = (thi - tlo + 1) * 128
                    p = pbuf.pop(b)
                    for h01 in range(2):
                        h = 2 * hp + h01
                        nc.tensor.matmul(
                            ctx_ps[h01][:, tlo * 128:(thi + 1) * 128],
                            vv[b][:, h:h + 1, :], p[:, h01, :ncols],
                            start=(b == 4), stop=(b == B_ORDER[-1]),
                            skip_group_check=True)

                for i, b in enumerate(B_ORDER):
                    stage_st(b)
                    if i >= LAG:
                        stage_pv(B_ORDER[i - LAG])
                for b in B_ORDER[-LAG:]:
                    stage_pv(b)
                for h01 in range(2):
                    h = 2 * hp + h01
                    po = h01 * 64
                    stg = pR.tile([1, QR], f32, tag="sstg", name="sstg")
                    nc.scalar.copy(stg[:], ctx_ps[h01][64:65, :])
                    if h < 14:
                        nc.sync.dma_start(sumsA[h:h + 1, :], stg[:])
                    else:
                        nc.sync.dma_start(sumsB[h - 14:h - 13, :], stg[:])
                    nc.vector.tensor_copy(ctx[hp][po:po + 64, :],
                                          ctx_ps[h01][0:64, :])

            def normalize(hps, recb, h0):
                for hp in hps:
                    bc = pW.tile([128, QR], bf16, tag="bc", name="bc", bufs=2)
                    for h01 in range(2):
                        h, po = 2 * hp + h01, h01 * 64
                        rb0 = pR.tile([1, QR], bf16, tag="rb0", name="rb0")
                        nc.sync.dma_start(rb0[:], recb[h - h0:h - h0 + 1, :])
                        bch = pR.tile([64, QR], bf16, tag="bch", name="bch")
                        nc.gpsimd.partition_broadcast(bch[:], rb0[:])
                        nc.sync.dma_start(bc[po:po + 64, :], bch[:])
                    nc.vector.tensor_mul(ctx[hp][:], ctx[hp][:], bc[:])

            proj(0)
            proj(1)
            for hp in range(2, HP):
                proj(hp)
                attn(hp - 2)
            attn(HP - 2)
            # normalize heads 0..13 while attn(7) runs
            recbA = pW.tile([14, QR], bf16, tag="recbA", name="recbA")
            with nc.allow_low_precision(reason="softmax denom fits bf16"):
                nc.vector.reciprocal(recbA[:], sumsA[:])
            normalize(range(HP - 1), recbA, 0)
            attn(HP - 1)
            recbB = pW.tile([2, QR], bf16, tag="recbB", name="recbB")
            with nc.allow_low_precision(reason="softmax denom fits bf16"):
                nc.vector.reciprocal(recbB[:], sumsB[:])
            normalize([HP - 1], recbB, 14)

            # ---- output projection ----
            for ti in range(QT):
                ob = pW.tile([128, HID], f32, tag="ob", name="ob", bufs=3)
                for n2 in range(2):
                    o_ps = psA.tile([128, 512], f32, tag="pj", name="ops")
                    for f in range(FC):
                        nc.tensor.matmul(
                            o_ps[:], ctx[f][:, ti * 128:(ti + 1) * 128],
                            wo_t[f][:, n2 * 512:(n2 + 1) * 512],
                            start=(f == 0), stop=(f == FC - 1))
                    nc.any.tensor_copy(ob[:, n2 * 512:(n2 + 1) * 512],
                                       o_ps[:])
                nc.sync.dma_start(out_d[ti * 128:(ti + 1) * 128, :], ob[:])

    nc.compile()
    return nc


def _host_prep(x, wq, wk, wv, wo):
    bf = ml_dtypes.bfloat16
    xT = np.ascontiguousarray(x.T).astype(np.float32)  # [HID, T]
    wqT = np.ascontiguousarray((wq.astype(np.float32) * 0.125).T).astype(bf)
    wkT = np.ascontiguousarray(wk.T).astype(bf)
    wvT = np.ascontiguousarray(wv.T).astype(bf)
    woT = np.ascontiguousarray(wo.T).astype(bf)
    mlo = np.greater_equal.outer(np.arange(128), np.arange(128)).astype(bf)
    mhi = np.less_equal.outer(np.arange(128), np.arange(128)).astype(bf)

    inv_freq = ROPE_THETA ** (-np.arange(0, HD, 2, dtype=np.float64) / HD)  # [32]
    d_idx = np.arange(128) % HD
    freq_i = d_idx % 32
    sign = np.where(d_idx < 32, -1.0, 1.0)

    in_maps = []
    for c in range(NCORES):
        lo = c * QR - WINDOW
        xkv = np.zeros((HID, KV), np.float32)
        if lo < 0:
            xkv[:, -lo:] = xT[:, 0:lo + KV]
        else:
            xkv[:] = xT[:, lo:lo + KV]
        vones = np.ones((KV, 16), np.float32)
        if lo < 0:
            vones[0:-lo, :] = 0.0
        pos = lo + np.arange(KV, dtype=np.float64)  # [KV]
        ang = pos[None, :] * inv_freq[freq_i][:, None]  # [128, KV]
        rcos = np.cos(ang).astype(bf)
        rsin = (sign[:, None] * np.sin(ang)).astype(bf)
        in_maps.append({
            "xT": xkv.astype(bf),
            "wqT": wqT, "wkT": wkT, "wvT": wvT, "woT": woT,
            "vones": vones.astype(bf),
            "mlo": mlo, "mhi": mhi,
            "ropecos": rcos, "ropesin": rsin,
        })
    return in_maps


def _run(x, wq, wk, wv, wo, trace=False, tmpdir=None):
    from concourse.bass_utils import run_bass_kernel_spmd
    if "nc" not in _CACHE:
        _CACHE["nc"] = _build_program()
    nc = _CACHE["nc"]
    in_maps = _host_prep(x, wq, wk, wv, wo)
    res = run_bass_kernel_spmd(nc, in_maps, list(range(NCORES)),
                               trace=trace, tmpdir=tmpdir)
    out = np.concatenate([res.results[c]["out"] for c in range(NCORES)], axis=0)
    return np.ascontiguousarray(out).astype(np.float32), res


def kernel(x, wq, wk, wv, wo):
    out, _ = _run(x, wq, wk, wv, wo)
    return out


# revision 17
# speedup vs baseline: 1.2905x; 1.2905x over previous
"""Trainium2 Bass kernel for nn_MimiAttention (sliding-window causal attention).

Reference math (T=4096, HID=1024, 16 heads x 64 dims, window 512, RoPE):
  q = rope(x @ wq.T); k = rope(x @ wk.T); v = x @ wv.T
  ctx = sdpa(q, k, v, causal, local_window=(512, 0), scale=1/8)
  out = ctx @ wo.T

Sharding: sequence-parallel across 8 NeuronCores, zero communication.
Core c owns queries [c*512, (c+1)*512) and recomputes k/v over its kv
window [c*512-512, (c+1)*512) (halo recompute).

On-device layout: everything transposed (feature dim on partitions).
Softmax without max-subtraction (scores are small: |S/8| < ~4), row sums
via a ones-column appended to V, triangle masks as bf16 multiplies on
exp(S^T), all 16 head denominators inverted in one batched reciprocal.
RoPE in the transposed layout: partner-dim swap via 4 small SBUF->SBUF
DMAs (partition shift), then two table multiplies + add.
"""

import sys

sys.path.insert(0, "/opt/trn_rl_repo")

import numpy as np
import ml_dtypes

T, HID, NH, HD = 4096, 1024, 16, 64
WINDOW = 512
ROPE_THETA = 10000.0
NCORES = 8
QR = T // NCORES          # 512 queries per core
KV = QR + WINDOW          # 1024 kv rows per core (incl. halo)
NB = KV // 128            # 8 kv blocks
QT = QR // 128            # 4 query tiles
HP = NH // 2              # 8 head pairs
FC = HID // 128           # 8 feature chunks

_CACHE = {}


def _build_program():
    import concourse.mybir as mybir
    import concourse.tile as tile
    from concourse import bacc

    f32 = mybir.dt.float32
    bf16 = mybir.dt.bfloat16
    Exp = mybir.ActivationFunctionType.Exp

    nc = bacc.Bacc("TRN2", target_bir_lowering=False, debug=False,
                   num_devices=NCORES)

    xT_d = nc.declare_dram_parameter("xT", [HID, KV], bf16, isOutput=False)
    wqT_d = nc.declare_dram_parameter("wqT", [HID, HID], bf16, isOutput=False)
    wkT_d = nc.declare_dram_parameter("wkT", [HID, HID], bf16, isOutput=False)
    wvT_d = nc.declare_dram_parameter("wvT", [HID, HID], bf16, isOutput=False)
    woT_d = nc.declare_dram_parameter("woT", [HID, HID], bf16, isOutput=False)
    vones_d = nc.declare_dram_parameter("vones", [KV, 16], bf16, isOutput=False)
    mlo_d = nc.declare_dram_parameter("mlo", [128, 128], bf16, isOutput=False)
    mhi_d = nc.declare_dram_parameter("mhi", [128, 128], bf16, isOutput=False)
    rc_d = nc.declare_dram_parameter("ropecos", [128, KV], bf16, isOutput=False)
    rs_d = nc.declare_dram_parameter("ropesin", [128, KV], bf16, isOutput=False)
    out_d = nc.declare_dram_parameter("out", [QR, HID], f32, isOutput=True)

    with tile.TileContext(nc) as tc:
        with (
            tc.tile_pool(name="const", bufs=1) as cpool,
            tc.tile_pool(name="psA", bufs=3, space="PSUM") as psA,
            tc.tile_pool(name="psS", bufs=3, space="PSUM") as psS,
            tc.tile_pool(name="psC", bufs=2, space="PSUM") as psC,
            tc.tile_pool(name="pP", bufs=4) as pP,
            tc.tile_pool(name="pR", bufs=3) as pR,
            tc.tile_pool(name="pW", bufs=2) as pW,
        ):
            # ---- constants / weights into SBUF (xt/wv first for v-proj) ----
            xt, wv_t = [], []
            for f in range(FC):
                t_ = cpool.tile([128, KV], bf16, tag=f"xt{f}", name=f"xt{f}")
                nc.sync.dma_start(t_[:], xT_d[f * 128:(f + 1) * 128, :])
                xt.append(t_)
                t_ = cpool.tile([128, HID], bf16, tag=f"wv{f}", name=f"wv{f}")
                nc.sync.dma_start(t_[:], wvT_d[f * 128:(f + 1) * 128, :])
                wv_t.append(t_)

            def load_rows(dram, n_free, tagp):
                ts_ = []
                for f in range(FC):
                    t_ = cpool.tile([128, n_free], bf16, tag=f"{tagp}{f}",
                                    name=f"{tagp}{f}")
                    nc.sync.dma_start(t_[:], dram[f * 128:(f + 1) * 128, :])
                    ts_.append(t_)
                return ts_

            wq_t = load_rows(wqT_d, HID, "wq")
            rc = cpool.tile([128, KV], bf16, tag="rc", name="rc")
            nc.sync.dma_start(rc[:], rc_d[:])
            rs = cpool.tile([128, KV], bf16, tag="rs", name="rs")
            nc.sync.dma_start(rs[:], rs_d[:])
            wk_t = load_rows(wkT_d, HID, "wk")
            mlo = cpool.tile([128, 128], bf16, tag="mlo", name="mlo")
            nc.sync.dma_start(mlo[:], mlo_d[:])
            mhi = cpool.tile([128, 128], bf16, tag="mhi", name="mhi")
            nc.sync.dma_start(mhi[:], mhi_d[:])
            wo_t = load_rows(woT_d, HID, "wo")

            qT = [cpool.tile([128, QR], bf16, tag=f"qT{h}", name=f"qT{h}")
                  for h in range(HP)]
            kT = [cpool.tile([128, KV], bf16, tag=f"kT{h}", name=f"kT{h}")
                  for h in range(HP)]
            vv = [cpool.tile([128, 16, 65], bf16, tag=f"vv{b}", name=f"vv{b}")
                  for b in range(NB)]
            ctx = [cpool.tile([128, QR], bf16, tag=f"ctx{h}", name=f"ctx{h}")
                   for h in range(HP)]
            sumsA = cpool.tile([14, QR], f32, tag="sumsA", name="sumsA")
            sumsB = cpool.tile([2, QR], f32, tag="sumsB", name="sumsB")

            # ---- v projection (+ ones column) ----
            for rb in range(NB):
                nc.sync.dma_start(vv[rb][:, :, 64:65],
                                  vones_d[rb * 128:(rb + 1) * 128, :])
                for d2 in range(2):
                    v_ps = psA.tile([128, 8, 64], f32, tag="pj", name="vps")
                    for f in range(FC):
                        nc.tensor.matmul(
                            v_ps[:], xt[f][:, rb * 128:(rb + 1) * 128],
                            wv_t[f][:, d2 * 512:(d2 + 1) * 512],
                            start=(f == 0), stop=(f == FC - 1))
                    nc.vector.tensor_copy(vv[rb][:, d2 * 8:(d2 + 1) * 8, 0:64],
                                          v_ps[:])

            # ---- RoPE: dst[:, dc0:dc0+512] = rope(src_ps) ----
            def rope_apply(src_ps, dst, tc0, dc0):
                n = 512
                raw = pR.tile([128, n], bf16, tag="rraw", name="rraw")
                nc.vector.tensor_copy(raw[:], src_ps[:])
                swp = pR.tile([128, n], bf16, tag="rswp", name="rswp")
                for g in range(4):
                    pg = (g ^ 1) * 32
                    nc.sync.dma_start(swp[g * 32:(g + 1) * 32, :],
                                      raw[pg:pg + 32, :])
                nc.vector.tensor_mul(dst[:, dc0:dc0 + n], raw[:],
                                     rc[:, tc0:tc0 + n])
                t2 = pR.tile([128, n], bf16, tag="rt2", name="rt2")
                nc.vector.tensor_mul(t2[:], swp[:], rs[:, tc0:tc0 + n])
                nc.vector.tensor_add(dst[:, dc0:dc0 + n],
                                     dst[:, dc0:dc0 + n], t2[:])

            # ---- q^T / k^T projections with RoPE ----
            def proj(hp):
                q_ps = psA.tile([128, QR], f32, tag="pj", name="qps")
                for f in range(FC):
                    nc.tensor.matmul(
                        q_ps[:], wq_t[f][:, hp * 128:(hp + 1) * 128],
                        xt[f][:, WINDOW:KV],
                        start=(f == 0), stop=(f == FC - 1))
                rope_apply(q_ps, qT[hp], WINDOW, 0)
                for rh in range(2):
                    k_ps = psA.tile([128, 512], f32, tag="pj", name="kps")
                    for f in range(FC):
                        nc.tensor.matmul(
                            k_ps[:], wk_t[f][:, hp * 128:(hp + 1) * 128],
                            xt[f][:, rh * 512:(rh + 1) * 512],
                            start=(f == 0), stop=(f == FC - 1))
                    rope_apply(k_ps, kT[hp], rh * 512, rh * 512)

            # ---- attention for one head pair (h0/h1 share st/p tiles) ----
            B_ORDER = [4, 5, 6, 7, 0, 1, 2, 3]  # b=4 first: full-width write
            LAG = 2

            def attn(hp):
                ctx_ps = [psC.tile([65, QR], f32, tag="ctx", name="ctxps")
                          for _ in range(2)]
                pbuf = {}

                def stage_st(b):
                    tlo, thi = max(0, b - 4), min(QT - 1, b)
                    ncols = (thi - tlo + 1) * 128
                    p = pP.tile([128, 2, 512], bf16, tag="p", name="p")
                    sts = []
                    for h01 in range(2):
                        po = h01 * 64
                        st = psS.tile([128, 512], f32, tag="st", name="st")
                        nc.tensor.matmul(
                            st[:, :ncols],
                            kT[hp][po:po + 64, b * 128:(b + 1) * 128],
                            qT[hp][po:po + 64, tlo * 128:(thi + 1) * 128],
                            start=True, stop=True, tile_position=(po, 0))
                        sts.append(st)
                    for h01 in range(2):
                        nc.scalar.activation(p[:, h01, :ncols],
                                             sts[h01][:, :ncols], Exp)
                    if b <= QT - 1:
                        c0 = (b - tlo) * 128
                        for h01 in range(2):
                            nc.vector.tensor_mul(p[:, h01, c0:c0 + 128],
                                                 p[:, h01, c0:c0 + 128], mlo[:])
                    if b >= 4:
                        for h01 in range(2):
                            nc.vector.tensor_mul(p[:, h01, 0:128],
                                                 p[:, h01, 0:128], mhi[:])
                    pbuf[b] = p

                def stage_pv(b):
                    tlo, thi = max(0, b - 4), min(QT - 1, b)
                    ncols = (thi - tlo + 1) * 128
                    p = pbuf.pop(b)
                    for h01 in range(2):
                        h = 2 * hp + h01
                        nc.tensor.matmul(
                            ctx_ps[h01][:, tlo * 128:(thi + 1) * 128],
                            vv[b][:, h:h + 1, :], p[:, h01, :ncols],
                            start=(b == 4), stop=(b == B_ORDER[-1]),
                            skip_group_check=True)

                for i, b in enumerate(B_ORDER):
                    stage_st(b)
                    if i >= LAG:
                        stage_pv(B_ORDER[i - LAG])
                for b in B_ORDER[-LAG:]:
                    stage_pv(b)
                for h01 in range(2):
                    h = 2 * hp + h01
                    po = h01 * 64
                    stg = pR.tile([1, QR], f32, tag="sstg", name="sstg")
                    nc.scalar.copy(stg[:], ctx_ps[h01][64:65, :])
                    if h < 14:
                        nc.sync.dma_start(sumsA[h:h + 1, :], stg[:])
                    else:
                        nc.sync.dma_start(sumsB[h - 14:h - 13, :], stg[:])
                    nc.vector.tensor_copy(ctx[hp][po:po + 64, :],
                                          ctx_ps[h01][0:64, :])

            def normalize(hps, recb, h0):
                for hp in hps:
                    bc = pW.tile([128, QR], bf16, tag="bc", name="bc", bufs=2)
                    for h01 in range(2):
                        h, po = 2 * hp + h01, h01 * 64
                        rb0 = pR.tile([1, QR], bf16, tag="rb0", name="rb0")
                        nc.sync.dma_start(rb0[:], recb[h - h0:h - h0 + 1, :])
                        bch = pR.tile([64, QR], bf16, tag="bch", name="bch")
                        nc.gpsimd.partition_broadcast(bch[:], rb0[:])
                        nc.sync.dma_start(bc[po:po + 64, :], bch[:])
                    nc.vector.tensor_mul(ctx[hp][:], ctx[hp][:], bc[:])

            proj(0)
            proj(1)
            for hp in range(2, HP):
                proj(hp)
                attn(hp - 2)
            attn(HP - 2)
            # normalize heads 0..13 while attn(7) runs
            recbA = pW.tile([14, QR], bf16, tag="recbA", name="recbA")
            with nc.allow_low_precision(reason="softmax denom fits bf16"):
                nc.vector.reciprocal(recbA[:], sumsA[:])
            normalize(range(HP - 1), recbA, 0)
            attn(HP - 1)
            recbB = pW.tile([2, QR], bf16, tag="recbB", name="recbB")
            with nc.allow_low_precision(reason="softmax denom fits bf16"):
                nc.vector.reciprocal(recbB[:], sumsB[:])
            normalize([HP - 1], recbB, 14)

            # ---- output projection ----
            for ti in range(QT):
                ob = pW.tile([128, HID], f32, tag="ob", name="ob", bufs=3)
                for n2 in range(2):
                    o_ps = psA.tile([128, 512], f32, tag="pj", name="ops")
                    for f in range(FC):
                        nc.tensor.matmul(
                            o_ps[:], ctx[f][:, ti * 128:(ti + 1) * 128],
                            wo_t[f][:, n2 * 512:(n2 + 1) * 512],
                            start=(f == 0), stop=(f == FC - 1))
                    nc.any.tensor_copy(ob[:, n2 * 512:(n2 + 1) * 512],
                                       o_ps[:])
                nc.sync.dma_start(out_d[ti * 128:(ti + 1) * 128, :], ob[:])

    nc.compile()
    return nc


def _host_prep(x, wq, wk, wv, wo):
    bf = ml_dtypes.bfloat16
    xT = np.ascontiguousarray(x.T).astype(np.float32)  # [HID, T]
    wqT = np.ascontiguousarray((wq.astype(np.float32) * 0.125).T).astype(bf)
    wkT = np.ascontiguousarray(wk.T).astype(bf)
    wvT = np.ascontiguousarray(wv.T).astype(bf)
    woT = np.ascontiguousarray(wo.T).astype(bf)
    mlo = np.greater_equal.outer(np.arange(128), np.arange(128)).astype(bf)
    mhi = np.less_equal.outer(np.arange(128), np.arange(128)).astype(bf)

    inv_freq = ROPE_THETA ** (-np.arange(0, HD, 2, dtype=np.float64) / HD)  # [32]
    d_idx = np.arange(128) % HD
    freq_i = d_idx % 32
    sign = np.where(d_idx < 32, -1.0, 1.0)

    in_maps = []
    for c in range(NCORES):
        lo = c * QR - WINDOW
        xkv = np.zeros((HID, KV), np.float32)
        if lo < 0:
            xkv[:, -lo:] = xT[:, 0:lo + KV]
        else:
            xkv[:] = xT[:, lo:lo + KV]
        vones = np.ones((KV, 16), np.float32)
        if lo < 0:
            vones[0:-lo, :] = 0.0
        pos = lo + np.arange(KV, dtype=np.float64)  # [KV]
        ang = pos[None, :] * inv_freq[freq_i][:, None]  # [128, KV]
        rcos = np.cos(ang).astype(bf)
        rsin = (sign[:, None] * np.sin(ang)).astype(bf)
        in_maps.append({
            "xT": xkv.astype(bf),
            "wqT": wqT, "wkT": wkT, "wvT": wvT, "woT": woT,
            "vones": vones.astype(bf),
            "mlo": mlo, "mhi": mhi,
            "ropecos": rcos, "ropesin": rsin,
        })
    return in_maps


def _run(x, wq, wk, wv, wo, trace=False, tmpdir=None):
    from concourse.bass_utils import run_bass_kernel_spmd
    if "nc" not in _CACHE:
        _CACHE["nc"] = _build_program()
    nc = _CACHE["nc"]
    in_maps = _host_prep(x, wq, wk, wv, wo)
    res = run_bass_kernel_spmd(nc, in_maps, list(range(NCORES)),
                               trace=trace, tmpdir=tmpdir)
    out = np.concatenate([res.results[c]["out"] for c in range(NCORES)], axis=0)
    return np.ascontiguousarray(out).astype(np.float32), res


def kernel(x, wq, wk, wv, wo):
    out, _ = _run(x, wq, wk, wv, wo)
    return out


# revision 18
# speedup vs baseline: 1.5451x; 1.1973x over previous
"""Trainium2 Bass kernel for nn_MimiAttention (sliding-window causal attention).

Reference math (T=4096, HID=1024, 16 heads x 64 dims, window 512, RoPE):
  q = rope(x @ wq.T); k = rope(x @ wk.T); v = x @ wv.T
  ctx = sdpa(q, k, v, causal, local_window=(512, 0), scale=1/8)
  out = ctx @ wo.T

Sharding: sequence-parallel across 8 NeuronCores, zero communication.
Core c owns queries [c*512, (c+1)*512) and recomputes k/v over its kv
window [c*512-512, (c+1)*512) (halo recompute).

On-device layout: everything transposed (feature dim on partitions).
Softmax without max-subtraction (scores are small: |S/8| < ~4), row sums
via a ones-column appended to V, triangle masks as bf16 multiplies on
exp(S^T), all 16 head denominators inverted in one batched reciprocal.
RoPE in the transposed layout: partner-dim swap via 4 small SBUF->SBUF
DMAs (partition shift), then two table multiplies + add.
"""

import sys

sys.path.insert(0, "/opt/trn_rl_repo")

import numpy as np
import ml_dtypes

T, HID, NH, HD = 4096, 1024, 16, 64
WINDOW = 512
ROPE_THETA = 10000.0
NCORES = 8
QR = T // NCORES          # 512 queries per core
KV = QR + WINDOW          # 1024 kv rows per core (incl. halo)
NB = KV // 128            # 8 kv blocks
QT = QR // 128            # 4 query tiles
HP = NH // 2              # 8 head pairs
FC = HID // 128           # 8 feature chunks

_CACHE = {}


def _build_program():
    import concourse.mybir as mybir
    import concourse.tile as tile
    from concourse import bacc

    f32 = mybir.dt.float32
    bf16 = mybir.dt.bfloat16
    Exp = mybir.ActivationFunctionType.Exp

    nc = bacc.Bacc("TRN2", target_bir_lowering=False, debug=False,
                   num_devices=NCORES)

    xT_d = nc.declare_dram_parameter("xT", [HID, KV], bf16, isOutput=False)
    wqT_d = nc.declare_dram_parameter("wqT", [HID, HID], bf16, isOutput=False)
    wkT_d = nc.declare_dram_parameter("wkT", [HID, HID], bf16, isOutput=False)
    wvT_d = nc.declare_dram_parameter("wvT", [HID, HID], bf16, isOutput=False)
    woT_d = nc.declare_dram_parameter("woT", [HID, HID], bf16, isOutput=False)
    vones_d = nc.declare_dram_parameter("vones", [KV, 16], bf16, isOutput=False)
    mlo_d = nc.declare_dram_parameter("mlo", [128, 128], bf16, isOutput=False)
    mhi_d = nc.declare_dram_parameter("mhi", [128, 128], bf16, isOutput=False)
    rc_d = nc.declare_dram_parameter("ropecos", [128, KV], bf16, isOutput=False)
    rs_d = nc.declare_dram_parameter("ropesin", [128, KV], bf16, isOutput=False)
    out_d = nc.declare_dram_parameter("out", [QR, HID], f32, isOutput=True)

    with tile.TileContext(nc) as tc:
        with (
            tc.tile_pool(name="const", bufs=1) as cpool,
            tc.tile_pool(name="psA", bufs=2, space="PSUM") as psA,
            tc.tile_pool(name="psS", bufs=4, space="PSUM") as psS,
            tc.tile_pool(name="psC", bufs=2, space="PSUM") as psC,
            tc.tile_pool(name="pP", bufs=6) as pP,
            tc.tile_pool(name="pR", bufs=3) as pR,
            tc.tile_pool(name="pW", bufs=2) as pW,
        ):
            # ---- constants / weights into SBUF (xt/wv first for v-proj) ----
            xt, wv_t = [], []
            for f in range(FC):
                t_ = cpool.tile([128, KV], bf16, tag=f"xt{f}", name=f"xt{f}")
                nc.sync.dma_start(t_[:], xT_d[f * 128:(f + 1) * 128, :])
                xt.append(t_)
                t_ = cpool.tile([128, HID], bf16, tag=f"wv{f}", name=f"wv{f}")
                nc.sync.dma_start(t_[:], wvT_d[f * 128:(f + 1) * 128, :])
                wv_t.append(t_)

            def load_rows(dram, n_free, tagp):
                ts_ = []
                for f in range(FC):
                    t_ = cpool.tile([128, n_free], bf16, tag=f"{tagp}{f}",
                                    name=f"{tagp}{f}")
                    nc.sync.dma_start(t_[:], dram[f * 128:(f + 1) * 128, :])
                    ts_.append(t_)
                return ts_

            wq_t = load_rows(wqT_d, HID, "wq")
            rc = cpool.tile([128, KV], bf16, tag="rc", name="rc")
            nc.sync.dma_start(rc[:], rc_d[:])
            rs = cpool.tile([128, KV], bf16, tag="rs", name="rs")
            nc.sync.dma_start(rs[:], rs_d[:])
            wk_t = load_rows(wkT_d, HID, "wk")
            mlo = cpool.tile([128, 128], bf16, tag="mlo", name="mlo")
            nc.sync.dma_start(mlo[:], mlo_d[:])
            mhi = cpool.tile([128, 128], bf16, tag="mhi", name="mhi")
            nc.sync.dma_start(mhi[:], mhi_d[:])
            wo_t = load_rows(woT_d, HID, "wo")

            qT = [cpool.tile([128, QR], bf16, tag=f"qT{h}", name=f"qT{h}")
                  for h in range(HP)]
            kT = [cpool.tile([128, KV], bf16, tag=f"kT{h}", name=f"kT{h}")
                  for h in range(HP)]
            vv = [cpool.tile([128, 16, 65], bf16, tag=f"vv{b}", name=f"vv{b}")
                  for b in range(NB)]
            ctx = [cpool.tile([128, QR], bf16, tag=f"ctx{h}", name=f"ctx{h}")
                   for h in range(HP)]
            sumsA = cpool.tile([14, QR], f32, tag="sumsA", name="sumsA")
            sumsB = cpool.tile([2, QR], f32, tag="sumsB", name="sumsB")

            # ---- v projection (+ ones column) ----
            for rb in range(NB):
                nc.sync.dma_start(vv[rb][:, :, 64:65],
                                  vones_d[rb * 128:(rb + 1) * 128, :])
                for d2 in range(2):
                    v_ps = psA.tile([128, 8, 64], f32, tag="pj", name="vps")
                    for f in range(FC):
                        nc.tensor.matmul(
                            v_ps[:], xt[f][:, rb * 128:(rb + 1) * 128],
                            wv_t[f][:, d2 * 512:(d2 + 1) * 512],
                            start=(f == 0), stop=(f == FC - 1))
                    nc.vector.tensor_copy(vv[rb][:, d2 * 8:(d2 + 1) * 8, 0:64],
                                          v_ps[:])

            # ---- RoPE: dst[:, dc0:dc0+512] = rope(src_ps) ----
            def rope_apply(src_ps, dst, tc0, dc0):
                n = 512
                raw = pR.tile([128, n], bf16, tag="rraw", name="rraw")
                nc.vector.tensor_copy(raw[:], src_ps[:])
                swp = pR.tile([128, n], bf16, tag="rswp", name="rswp")
                for g in range(4):
                    pg = (g ^ 1) * 32
                    nc.sync.dma_start(swp[g * 32:(g + 1) * 32, :],
                                      raw[pg:pg + 32, :])
                nc.vector.tensor_mul(dst[:, dc0:dc0 + n], raw[:],
                                     rc[:, tc0:tc0 + n])
                t2 = pR.tile([128, n], bf16, tag="rt2", name="rt2")
                nc.vector.tensor_mul(t2[:], swp[:], rs[:, tc0:tc0 + n])
                nc.vector.tensor_add(dst[:, dc0:dc0 + n],
                                     dst[:, dc0:dc0 + n], t2[:])

            # ---- q^T / k^T projections with RoPE ----
            def proj(hp):
                q_ps = psA.tile([128, QR], f32, tag="pj", name="qps")
                for f in range(FC):
                    nc.tensor.matmul(
                        q_ps[:], wq_t[f][:, hp * 128:(hp + 1) * 128],
                        xt[f][:, WINDOW:KV],
                        start=(f == 0), stop=(f == FC - 1))
                rope_apply(q_ps, qT[hp], WINDOW, 0)
                for rh in range(2):
                    k_ps = psA.tile([128, 512], f32, tag="pj", name="kps")
                    for f in range(FC):
                        nc.tensor.matmul(
                            k_ps[:], wk_t[f][:, hp * 128:(hp + 1) * 128],
                            xt[f][:, rh * 512:(rh + 1) * 512],
                            start=(f == 0), stop=(f == FC - 1))
                    rope_apply(k_ps, kT[hp], rh * 512, rh * 512)

            # ---- attention for one head pair (h0/h1 share st/p tiles) ----
            B_ORDER = [4, 5, 6, 7, 0, 1, 2, 3]  # b=4 first: full-width write
            LAG = 3

            def attn(hp):
                ctx_ps = [psC.tile([65, QR], f32, tag="ctx", name="ctxps")
                          for _ in range(2)]
                pbuf = {}

                def stage_st(b):
                    tlo, thi = max(0, b - 4), min(QT - 1, b)
                    ncols = (thi - tlo + 1) * 128
                    p = pP.tile([128, 2, 512], bf16, tag="p", name="p")
                    sts = []
                    for h01 in range(2):
                        po = h01 * 64
                        st = psS.tile([128, 512], f32, tag="st", name="st")
                        nc.tensor.matmul(
                            st[:, :ncols],
                            kT[hp][po:po + 64, b * 128:(b + 1) * 128],
                            qT[hp][po:po + 64, tlo * 128:(thi + 1) * 128],
                            start=True, stop=True, tile_position=(po, 0))
                        sts.append(st)
                    for h01 in range(2):
                        nc.scalar.activation(p[:, h01, :ncols],
                                             sts[h01][:, :ncols], Exp)
                    if b <= QT - 1:
                        c0 = (b - tlo) * 128
                        for h01 in range(2):
                            nc.vector.tensor_mul(p[:, h01, c0:c0 + 128],
                                                 p[:, h01, c0:c0 + 128], mlo[:])
                    if b >= 4:
                        for h01 in range(2):
                            nc.vector.tensor_mul(p[:, h01, 0:128],
                                                 p[:, h01, 0:128], mhi[:])
                    pbuf[b] = p

                def stage_pv(b):
                    tlo, thi = max(0, b - 4), min(QT - 1, b)
                    ncols = (thi - tlo + 1) * 128
                    p = pbuf.pop(b)
                    for h01 in range(2):
                        h = 2 * hp + h01
                        nc.tensor.matmul(
                            ctx_ps[h01][:, tlo * 128:(thi + 1) * 128],
                            vv[b][:, h:h + 1, :], p[:, h01, :ncols],
                            start=(b == 4), stop=(b == B_ORDER[-1]),
                            skip_group_check=True)

                for i, b in enumerate(B_ORDER):
                    stage_st(b)
                    if i >= LAG:
                        stage_pv(B_ORDER[i - LAG])
                for b in B_ORDER[-LAG:]:
                    stage_pv(b)
                for h01 in range(2):
                    h = 2 * hp + h01
                    po = h01 * 64
                    stg = pR.tile([1, QR], f32, tag="sstg", name="sstg")
                    nc.scalar.copy(stg[:], ctx_ps[h01][64:65, :])
                    if h < 14:
                        nc.sync.dma_start(sumsA[h:h + 1, :], stg[:])
                    else:
                        nc.sync.dma_start(sumsB[h - 14:h - 13, :], stg[:])
                    nc.vector.tensor_copy(ctx[hp][po:po + 64, :],
                                          ctx_ps[h01][0:64, :])

            def normalize(hps, recb, h0):
                for hp in hps:
                    bc = pW.tile([128, QR], bf16, tag="bc", name="bc", bufs=2)
                    for h01 in range(2):
                        h, po = 2 * hp + h01, h01 * 64
                        rb0 = pR.tile([1, QR], bf16, tag="rb0", name="rb0")
                        nc.sync.dma_start(rb0[:], recb[h - h0:h - h0 + 1, :])
                        bch = pR.tile([64, QR], bf16, tag="bch", name="bch")
                        nc.gpsimd.partition_broadcast(bch[:], rb0[:])
                        nc.sync.dma_start(bc[po:po + 64, :], bch[:])
                    nc.vector.tensor_mul(ctx[hp][:], ctx[hp][:], bc[:])

            proj(0)
            for hp in range(1, HP):
                proj(hp)
                attn(hp - 1)
            # normalize heads 0..13 while attn(7) runs
            recbA = pW.tile([14, QR], bf16, tag="recbA", name="recbA")
            with nc.allow_low_precision(reason="softmax denom fits bf16"):
                nc.vector.reciprocal(recbA[:], sumsA[:])
            normalize(range(HP - 1), recbA, 0)

            def o_partA(ti, n2):
                o_ps = psA.tile([128, 512], f32, tag="pj", name="ops")
                for f in range(FC - 1):
                    nc.tensor.matmul(
                        o_ps[:], ctx[f][:, ti * 128:(ti + 1) * 128],
                        wo_t[f][:, n2 * 512:(n2 + 1) * 512],
                        start=(f == 0), stop=(f == FC - 2))
                return o_ps

            def o_partB(ti, n2, o_ps, ob):
                f = FC - 1
                nc.tensor.matmul(
                    o_ps[:], ctx[f][:, ti * 128:(ti + 1) * 128],
                    wo_t[f][:, n2 * 512:(n2 + 1) * 512],
                    start=False, stop=True, skip_group_check=True)
                nc.any.tensor_copy(ob[:, n2 * 512:(n2 + 1) * 512], o_ps[:])

            held = {(0, 0): o_partA(0, 0), (0, 1): o_partA(0, 1)}
            attn(HP - 1)
            recbB = pW.tile([2, QR], bf16, tag="recbB", name="recbB")
            with nc.allow_low_precision(reason="softmax denom fits bf16"):
                nc.vector.reciprocal(recbB[:], sumsB[:])
            normalize([HP - 1], recbB, 14)

            # ---- output projection ----
            for ti in range(QT):
                ob = pW.tile([128, HID], f32, tag="ob", name="ob", bufs=3)
                for n2 in range(2):
                    if (ti, n2) in held:
                        o_partB(ti, n2, held.pop((ti, n2)), ob)
                    else:
                        o_ps = o_partA(ti, n2)
                        o_partB(ti, n2, o_ps, ob)
                nc.sync.dma_start(out_d[ti * 128:(ti + 1) * 128, :], ob[:])

    nc.compile()
    return nc


def _host_prep(x, wq, wk, wv, wo):
    bf = ml_dtypes.bfloat16
    xT = np.ascontiguousarray(x.T).astype(np.float32)  # [HID, T]
    wqT = np.ascontiguousarray((wq.astype(np.float32) * 0.125).T).astype(bf)
    wkT = np.ascontiguousarray(wk.T).astype(bf)
    wvT = np.ascontiguousarray(wv.T).astype(bf)
    woT = np.ascontiguousarray(wo.T).astype(bf)
    mlo = np.greater_equal.outer(np.arange(128), np.arange(128)).astype(bf)
    mhi = np.less_equal.outer(np.arange(128), np.arange(128)).astype(bf)

    inv_freq = ROPE_THETA ** (-np.arange(0, HD, 2, dtype=np.float64) / HD)  # [32]
    d_idx = np.arange(128) % HD
    freq_i = d_idx % 32
    sign = np.where(d_idx < 32, -1.0, 1.0)

    in_maps = []
    for c in range(NCORES):
        lo = c * QR - WINDOW
        xkv = np.zeros((HID, KV), np.float32)
        if lo < 0:
            xkv[:, -lo:] = xT[:, 0:lo + KV]
        else:
            xkv[:] = xT[:, lo:lo + KV]
        vones = np.ones((KV, 16), np.float32)
        if lo < 0:
            vones[0:-lo, :] = 0.0
        pos = lo + np.arange(KV, dtype=np.float64)  # [KV]
        ang = pos[None, :] * inv_freq[freq_i][:, None]  # [128, KV]
        rcos = np.cos(ang).astype(bf)
        rsin = (sign[:, None] * np.sin(ang)).astype(bf)
        in_maps.append({
            "xT": xkv.astype(bf),
            "wqT": wqT, "wkT": wkT, "wvT": wvT, "woT": woT,
            "vones": vones.astype(bf),
            "mlo": mlo, "mhi": mhi,
            "ropecos": rcos, "ropesin": rsin,
        })
    return in_maps


def _run(x, wq, wk, wv, wo, trace=False, tmpdir=None):
    from concourse.bass_utils import run_bass_kernel_spmd
    if "nc" not in _CACHE:
        _CACHE["nc"] = _build_program()
    nc = _CACHE["nc"]
    in_maps = _host_prep(x, wq, wk, wv, wo)
    res = run_bass_kernel_spmd(nc, in_maps, list(range(NCORES)),
                               trace=trace, tmpdir=tmpdir)
    out = np.concatenate([res.results[c]["out"] for c in range(NCORES)], axis=0)
    return np.ascontiguousarray(out).astype(np.float32), res


def kernel(x, wq, wk, wv, wo):
    out, _ = _run(x, wq, wk, wv, wo)
    return out


# revision 19
# speedup vs baseline: 1.5533x; 1.0053x over previous
"""Trainium2 Bass kernel for nn_MimiAttention (sliding-window causal attention).

Reference math (T=4096, HID=1024, 16 heads x 64 dims, window 512, RoPE):
  q = rope(x @ wq.T); k = rope(x @ wk.T); v = x @ wv.T
  ctx = sdpa(q, k, v, causal, local_window=(512, 0), scale=1/8)
  out = ctx @ wo.T

Sharding: sequence-parallel across 8 NeuronCores, zero communication.
Core c owns queries [c*512, (c+1)*512) and recomputes k/v over its kv
window [c*512-512, (c+1)*512) (halo recompute).

On-device layout: everything transposed (feature dim on partitions).
Softmax without max-subtraction (scores are small: |S/8| < ~4), row sums
via a ones-column appended to V, triangle masks as bf16 multiplies on
exp(S^T), all 16 head denominators inverted in one batched reciprocal.
RoPE in the transposed layout: partner-dim swap via 4 small SBUF->SBUF
DMAs (partition shift), then two table multiplies + add.
"""

import sys

sys.path.insert(0, "/opt/trn_rl_repo")

import numpy as np
import ml_dtypes

T, HID, NH, HD = 4096, 1024, 16, 64
WINDOW = 512
ROPE_THETA = 10000.0
NCORES = 8
QR = T // NCORES          # 512 queries per core
KV = QR + WINDOW          # 1024 kv rows per core (incl. halo)
NB = KV // 128            # 8 kv blocks
QT = QR // 128            # 4 query tiles
HP = NH // 2              # 8 head pairs
FC = HID // 128           # 8 feature chunks

_CACHE = {}


def _build_program():
    import concourse.mybir as mybir
    import concourse.tile as tile
    from concourse import bacc

    f32 = mybir.dt.float32
    bf16 = mybir.dt.bfloat16
    Exp = mybir.ActivationFunctionType.Exp

    nc = bacc.Bacc("TRN2", target_bir_lowering=False, debug=False,
                   num_devices=NCORES)

    xT_d = nc.declare_dram_parameter("xT", [HID, KV], bf16, isOutput=False)
    wqT_d = nc.declare_dram_parameter("wqT", [HID, HID], bf16, isOutput=False)
    wkT_d = nc.declare_dram_parameter("wkT", [HID, HID], bf16, isOutput=False)
    wvT_d = nc.declare_dram_parameter("wvT", [HID, HID], bf16, isOutput=False)
    woT_d = nc.declare_dram_parameter("woT", [HID, HID], bf16, isOutput=False)
    vones_d = nc.declare_dram_parameter("vones", [KV, 16], bf16, isOutput=False)
    mlo_d = nc.declare_dram_parameter("mlo", [128, 128], bf16, isOutput=False)
    mhi_d = nc.declare_dram_parameter("mhi", [128, 128], bf16, isOutput=False)
    rc_d = nc.declare_dram_parameter("ropecos", [128, KV], bf16, isOutput=False)
    rs_d = nc.declare_dram_parameter("ropesin", [128, KV], bf16, isOutput=False)
    out_d = nc.declare_dram_parameter("out", [QR, HID], f32, isOutput=True)

    with tile.TileContext(nc) as tc:
        with (
            tc.tile_pool(name="const", bufs=1) as cpool,
            tc.tile_pool(name="psA", bufs=2, space="PSUM") as psA,
            tc.tile_pool(name="psS", bufs=4, space="PSUM") as psS,
            tc.tile_pool(name="psC", bufs=2, space="PSUM") as psC,
            tc.tile_pool(name="pP", bufs=6) as pP,
            tc.tile_pool(name="pR", bufs=3) as pR,
            tc.tile_pool(name="pW", bufs=2) as pW,
        ):
            # ---- constants / weights into SBUF (xt/wv first for v-proj) ----
            xt, wv_t = [], []
            for f in range(FC):
                t_ = cpool.tile([128, KV], bf16, tag=f"xt{f}", name=f"xt{f}")
                nc.sync.dma_start(t_[:], xT_d[f * 128:(f + 1) * 128, :])
                xt.append(t_)
                t_ = cpool.tile([128, HID], bf16, tag=f"wv{f}", name=f"wv{f}")
                nc.sync.dma_start(t_[:], wvT_d[f * 128:(f + 1) * 128, :])
                wv_t.append(t_)

            def load_rows(dram, n_free, tagp):
                ts_ = []
                for f in range(FC):
                    t_ = cpool.tile([128, n_free], bf16, tag=f"{tagp}{f}",
                                    name=f"{tagp}{f}")
                    nc.sync.dma_start(t_[:], dram[f * 128:(f + 1) * 128, :])
                    ts_.append(t_)
                return ts_

            wq_t = load_rows(wqT_d, HID, "wq")
            rc = cpool.tile([128, KV], bf16, tag="rc", name="rc")
            nc.sync.dma_start(rc[:], rc_d[:])
            rs = cpool.tile([128, KV], bf16, tag="rs", name="rs")
            nc.sync.dma_start(rs[:], rs_d[:])
            wk_t = load_rows(wkT_d, HID, "wk")
            mlo = cpool.tile([128, 128], bf16, tag="mlo", name="mlo")
            nc.sync.dma_start(mlo[:], mlo_d[:])
            mhi = cpool.tile([128, 128], bf16, tag="mhi", name="mhi")
            nc.sync.dma_start(mhi[:], mhi_d[:])
            wo_t = load_rows(woT_d, HID, "wo")

            qT = [cpool.tile([128, QR], bf16, tag=f"qT{h}", name=f"qT{h}")
                  for h in range(HP)]
            kT = [cpool.tile([128, KV], bf16, tag=f"kT{h}", name=f"kT{h}")
                  for h in range(HP)]
            vv = [cpool.tile([128, 16, 65], bf16, tag=f"vv{b}", name=f"vv{b}")
                  for b in range(NB)]
            ctx = [cpool.tile([128, QR], bf16, tag=f"ctx{h}", name=f"ctx{h}")
                   for h in range(HP)]
            sumsA = cpool.tile([14, QR], f32, tag="sumsA", name="sumsA")
            sumsB = cpool.tile([2, QR], f32, tag="sumsB", name="sumsB")

            # ---- v projection (+ ones column) ----
            for rb in range(NB):
                nc.sync.dma_start(vv[rb][:, :, 64:65],
                                  vones_d[rb * 128:(rb + 1) * 128, :])
                for d2 in range(2):
                    v_ps = psA.tile([128, 8, 64], f32, tag="pj", name="vps")
                    for f in range(FC):
                        nc.tensor.matmul(
                            v_ps[:], xt[f][:, rb * 128:(rb + 1) * 128],
                            wv_t[f][:, d2 * 512:(d2 + 1) * 512],
                            start=(f == 0), stop=(f == FC - 1))
                    nc.vector.tensor_copy(vv[rb][:, d2 * 8:(d2 + 1) * 8, 0:64],
                                          v_ps[:])

            # ---- RoPE: dst[:, dc0:dc0+512] = rope(src_ps) ----
            def rope_apply(src_ps, dst, tc0, dc0):
                n = 512
                raw = pR.tile([128, n], bf16, tag="rraw", name="rraw")
                nc.vector.tensor_copy(raw[:], src_ps[:])
                swp = pR.tile([128, n], bf16, tag="rswp", name="rswp")
                for g in range(4):
                    pg = (g ^ 1) * 32
                    nc.sync.dma_start(swp[g * 32:(g + 1) * 32, :],
                                      raw[pg:pg + 32, :])
                nc.vector.tensor_mul(dst[:, dc0:dc0 + n], raw[:],
                                     rc[:, tc0:tc0 + n])
                t2 = pR.tile([128, n], bf16, tag="rt2", name="rt2")
                nc.vector.tensor_mul(t2[:], swp[:], rs[:, tc0:tc0 + n])
                nc.vector.tensor_add(dst[:, dc0:dc0 + n],
                                     dst[:, dc0:dc0 + n], t2[:])

            # ---- q^T / k^T projections with RoPE ----
            def proj(hp):
                q_ps = psA.tile([128, QR], f32, tag="pj", name="qps")
                for f in range(FC):
                    nc.tensor.matmul(
                        q_ps[:], wq_t[f][:, hp * 128:(hp + 1) * 128],
                        xt[f][:, WINDOW:KV],
                        start=(f == 0), stop=(f == FC - 1))
                rope_apply(q_ps, qT[hp], WINDOW, 0)
                for rh in range(2):
                    k_ps = psA.tile([128, 512], f32, tag="pj", name="kps")
                    for f in range(FC):
                        nc.tensor.matmul(
                            k_ps[:], wk_t[f][:, hp * 128:(hp + 1) * 128],
                            xt[f][:, rh * 512:(rh + 1) * 512],
                            start=(f == 0), stop=(f == FC - 1))
                    rope_apply(k_ps, kT[hp], rh * 512, rh * 512)

            # ---- attention for one head pair (h0/h1 share st/p tiles) ----
            B_ORDER = [4, 5, 6, 7, 0, 1, 2, 3]  # b=4 first: full-width write
            LAG = 2

            def attn(hp):
                ctx_ps = [psC.tile([65, QR], f32, tag="ctx", name="ctxps")
                          for _ in range(2)]
                pbuf = {}

                def stage_st(b):
                    tlo, thi = max(0, b - 4), min(QT - 1, b)
                    ncols = (thi - tlo + 1) * 128
                    p = pP.tile([128, 2, 512], bf16, tag="p", name="p")
                    sts = []
                    for h01 in range(2):
                        po = h01 * 64
                        st = psS.tile([128, 512], f32, tag="st", name="st")
                        nc.tensor.matmul(
                            st[:, :ncols],
                            kT[hp][po:po + 64, b * 128:(b + 1) * 128],
                            qT[hp][po:po + 64, tlo * 128:(thi + 1) * 128],
                            start=True, stop=True, tile_position=(po, 0))
                        sts.append(st)
                    for h01 in range(2):
                        nc.scalar.activation(p[:, h01, :ncols],
                                             sts[h01][:, :ncols], Exp)
                    if b <= QT - 1:
                        c0 = (b - tlo) * 128
                        for h01 in range(2):
                            nc.vector.tensor_mul(p[:, h01, c0:c0 + 128],
                                                 p[:, h01, c0:c0 + 128], mlo[:])
                    if b >= 4:
                        for h01 in range(2):
                            nc.vector.tensor_mul(p[:, h01, 0:128],
                                                 p[:, h01, 0:128], mhi[:])
                    pbuf[b] = p

                def stage_pv(b):
                    tlo, thi = max(0, b - 4), min(QT - 1, b)
                    ncols = (thi - tlo + 1) * 128
                    p = pbuf.pop(b)
                    for h01 in range(2):
                        h = 2 * hp + h01
                        nc.tensor.matmul(
                            ctx_ps[h01][:, tlo * 128:(thi + 1) * 128],
                            vv[b][:, h:h + 1, :], p[:, h01, :ncols],
                            start=(b == 4), stop=(b == B_ORDER[-1]),
                            skip_group_check=True)

                for i, b in enumerate(B_ORDER):
                    stage_st(b)
                    if i >= LAG:
                        stage_pv(B_ORDER[i - LAG])
                for b in B_ORDER[-LAG:]:
                    stage_pv(b)
                for h01 in range(2):
                    h = 2 * hp + h01
                    po = h01 * 64
                    stg = pR.tile([1, QR], f32, tag="sstg", name="sstg")
                    nc.scalar.copy(stg[:], ctx_ps[h01][64:65, :])
                    if h < 14:
                        nc.sync.dma_start(sumsA[h:h + 1, :], stg[:])
                    else:
                        nc.sync.dma_start(sumsB[h - 14:h - 13, :], stg[:])
                    nc.vector.tensor_copy(ctx[hp][po:po + 64, :],
                                          ctx_ps[h01][0:64, :])

            def normalize(hps, recb, h0):
                for hp in hps:
                    bc = pW.tile([128, QR], bf16, tag="bc", name="bc", bufs=2)
                    for h01 in range(2):
                        h, po = 2 * hp + h01, h01 * 64
                        rb0 = pR.tile([1, QR], bf16, tag="rb0", name="rb0")
                        nc.sync.dma_start(rb0[:], recb[h - h0:h - h0 + 1, :])
                        bch = pR.tile([64, QR], bf16, tag="bch", name="bch")
                        nc.gpsimd.partition_broadcast(bch[:], rb0[:])
                        nc.sync.dma_start(bc[po:po + 64, :], bch[:])
                    nc.vector.tensor_mul(ctx[hp][:], ctx[hp][:], bc[:])

            proj(0)
            for hp in range(1, HP):
                proj(hp)
                attn(hp - 1)
            # normalize heads 0..13 while attn(7) runs
            recbA = pW.tile([14, QR], bf16, tag="recbA", name="recbA")
            with nc.allow_low_precision(reason="softmax denom fits bf16"):
                nc.vector.reciprocal(recbA[:], sumsA[:])
            normalize(range(HP - 1), recbA, 0)

            def o_partA(ti, n2):
                o_ps = psA.tile([128, 512], f32, tag="pj", name="ops")
                for f in range(FC - 1):
                    nc.tensor.matmul(
                        o_ps[:], ctx[f][:, ti * 128:(ti + 1) * 128],
                        wo_t[f][:, n2 * 512:(n2 + 1) * 512],
                        start=(f == 0), stop=(f == FC - 2))
                return o_ps

            def o_partB(ti, n2, o_ps, ob):
                f = FC - 1
                nc.tensor.matmul(
                    o_ps[:], ctx[f][:, ti * 128:(ti + 1) * 128],
                    wo_t[f][:, n2 * 512:(n2 + 1) * 512],
                    start=False, stop=True, skip_group_check=True)
                nc.any.tensor_copy(ob[:, n2 * 512:(n2 + 1) * 512], o_ps[:])

            held = {(0, 0): o_partA(0, 0), (0, 1): o_partA(0, 1)}
            attn(HP - 1)
            recbB = pW.tile([2, QR], bf16, tag="recbB", name="recbB")
            with nc.allow_low_precision(reason="softmax denom fits bf16"):
                nc.vector.reciprocal(recbB[:], sumsB[:])
            normalize([HP - 1], recbB, 14)

            # ---- output projection ----
            for ti in range(QT):
                ob = pW.tile([128, HID], f32, tag="ob", name="ob", bufs=3)
                for n2 in range(2):
                    if (ti, n2) in held:
                        o_partB(ti, n2, held.pop((ti, n2)), ob)
                    else:
                        o_ps = o_partA(ti, n2)
                        o_partB(ti, n2, o_ps, ob)
                nc.sync.dma_start(out_d[ti * 128:(ti + 1) * 128, :], ob[:])

    nc.compile()
    return nc


def _host_prep(x, wq, wk, wv, wo):
    bf = ml_dtypes.bfloat16
    xT = np.ascontiguousarray(x.T).astype(np.float32)  # [HID, T]
    wqT = np.ascontiguousarray((wq.astype(np.float32) * 0.125).T).astype(bf)
    wkT = np.ascontiguousarray(wk.T).astype(bf)
    wvT = np.ascontiguousarray(wv.T).astype(bf)
    woT = np.ascontiguousarray(wo.T).astype(bf)
    mlo = np.greater_equal.outer(np.arange(128), np.arange(128)).astype(bf)
    mhi = np.less_equal.outer(np.arange(128), np.arange(128)).astype(bf)

    inv_freq = ROPE_THETA ** (-np.arange(0, HD, 2, dtype=np.float64) / HD)  # [32]
    d_idx = np.arange(128) % HD
    freq_i = d_idx % 32
    sign = np.where(d_idx < 32, -1.0, 1.0)

    in_maps = []
    for c in range(NCORES):
        lo = c * QR - WINDOW
        xkv = np.zeros((HID, KV), np.float32)
        if lo < 0:
            xkv[:, -lo:] = xT[:, 0:lo + KV]
        else:
            xkv[:] = xT[:, lo:lo + KV]
        vones = np.ones((KV, 16), np.float32)
        if lo < 0:
            vones[0:-lo, :] = 0.0
        pos = lo + np.arange(KV, dtype=np.float64)  # [KV]
        ang = pos[None, :] * inv_freq[freq_i][:, None]  # [128, KV]
        rcos = np.cos(ang).astype(bf)
        rsin = (sign[:, None] * np.sin(ang)).astype(bf)
        in_maps.append({
            "xT": xkv.astype(bf),
            "wqT": wqT, "wkT": wkT, "wvT": wvT, "woT": woT,
            "vones": vones.astype(bf),
            "mlo": mlo, "mhi": mhi,
            "ropecos": rcos, "ropesin": rsin,
        })
    return in_maps


def _run(x, wq, wk, wv, wo, trace=False, tmpdir=None):
    from concourse.bass_utils import run_bass_kernel_spmd
    if "nc" not in _CACHE:
        _CACHE["nc"] = _build_program()
    nc = _CACHE["nc"]
    in_maps = _host_prep(x, wq, wk, wv, wo)
    res = run_bass_kernel_spmd(nc, in_maps, list(range(NCORES)),
                               trace=trace, tmpdir=tmpdir)
    out = np.concatenate([res.results[c]["out"] for c in range(NCORES)], axis=0)
    return np.ascontiguousarray(out).astype(np.float32), res


def kernel(x, wq, wk, wv, wo):
    out, _ = _run(x, wq, wk, wv, wo)
    return out


# revision 20
# speedup vs baseline: 1.5618x; 1.0055x over previous
"""Trainium2 Bass kernel for nn_MimiAttention (sliding-window causal attention).

Reference math (T=4096, HID=1024, 16 heads x 64 dims, window 512, RoPE):
  q = rope(x @ wq.T); k = rope(x @ wk.T); v = x @ wv.T
  ctx = sdpa(q, k, v, causal, local_window=(512, 0), scale=1/8)
  out = ctx @ wo.T

Sharding: sequence-parallel across 8 NeuronCores, zero communication.
Core c owns queries [c*512, (c+1)*512) and recomputes k/v over its kv
window [c*512-512, (c+1)*512) (halo recompute).

On-device layout: everything transposed (feature dim on partitions).
Softmax without max-subtraction (scores are small: |S/8| < ~4), row sums
via a ones-column appended to V, triangle masks as bf16 multiplies on
exp(S^T), all 16 head denominators inverted in one batched reciprocal.
RoPE in the transposed layout: partner-dim swap via 4 small SBUF->SBUF
DMAs (partition shift), then two table multiplies + add.
"""

import sys

sys.path.insert(0, "/opt/trn_rl_repo")

import numpy as np
import ml_dtypes

T, HID, NH, HD = 4096, 1024, 16, 64
WINDOW = 512
ROPE_THETA = 10000.0
NCORES = 8
QR = T // NCORES          # 512 queries per core
KV = QR + WINDOW          # 1024 kv rows per core (incl. halo)
NB = KV // 128            # 8 kv blocks
QT = QR // 128            # 4 query tiles
HP = NH // 2              # 8 head pairs
FC = HID // 128           # 8 feature chunks

_CACHE = {}


def _build_program():
    import concourse.mybir as mybir
    import concourse.tile as tile
    from concourse import bacc

    f32 = mybir.dt.float32
    bf16 = mybir.dt.bfloat16
    Exp = mybir.ActivationFunctionType.Exp

    nc = bacc.Bacc("TRN2", target_bir_lowering=False, debug=False,
                   num_devices=NCORES)

    xT_d = nc.declare_dram_parameter("xT", [HID, KV], bf16, isOutput=False)
    wqT_d = nc.declare_dram_parameter("wqT", [HID, HID], bf16, isOutput=False)
    wkT_d = nc.declare_dram_parameter("wkT", [HID, HID], bf16, isOutput=False)
    wvT_d = nc.declare_dram_parameter("wvT", [HID, HID], bf16, isOutput=False)
    woT_d = nc.declare_dram_parameter("woT", [HID, HID], bf16, isOutput=False)
    vones_d = nc.declare_dram_parameter("vones", [KV, 16], bf16, isOutput=False)
    mlo_d = nc.declare_dram_parameter("mlo", [128, 128], bf16, isOutput=False)
    mhi_d = nc.declare_dram_parameter("mhi", [128, 128], bf16, isOutput=False)
    rc_d = nc.declare_dram_parameter("ropecos", [128, KV], bf16, isOutput=False)
    rs_d = nc.declare_dram_parameter("ropesin", [128, KV], bf16, isOutput=False)
    out_d = nc.declare_dram_parameter("out", [QR, HID], f32, isOutput=True)

    with tile.TileContext(nc) as tc:
        with (
            tc.tile_pool(name="const", bufs=1) as cpool,
            tc.tile_pool(name="psA", bufs=2, space="PSUM") as psA,
            tc.tile_pool(name="psS", bufs=4, space="PSUM") as psS,
            tc.tile_pool(name="psC", bufs=2, space="PSUM") as psC,
            tc.tile_pool(name="pP", bufs=6) as pP,
            tc.tile_pool(name="pR", bufs=3) as pR,
            tc.tile_pool(name="pW", bufs=2) as pW,
        ):
            # ---- constants / weights into SBUF (xt/wv first for v-proj) ----
            xt, wv_t = [], []
            for f in range(FC):
                t_ = cpool.tile([128, KV], bf16, tag=f"xt{f}", name=f"xt{f}")
                nc.sync.dma_start(t_[:], xT_d[f * 128:(f + 1) * 128, :])
                xt.append(t_)
                t_ = cpool.tile([128, HID], bf16, tag=f"wv{f}", name=f"wv{f}")
                nc.sync.dma_start(t_[:], wvT_d[f * 128:(f + 1) * 128, :])
                wv_t.append(t_)

            def load_rows(dram, n_free, tagp):
                ts_ = []
                for f in range(FC):
                    t_ = cpool.tile([128, n_free], bf16, tag=f"{tagp}{f}",
                                    name=f"{tagp}{f}")
                    nc.sync.dma_start(t_[:], dram[f * 128:(f + 1) * 128, :])
                    ts_.append(t_)
                return ts_

            wq_t = load_rows(wqT_d, HID, "wq")
            rc = cpool.tile([128, KV], bf16, tag="rc", name="rc")
            nc.sync.dma_start(rc[:], rc_d[:])
            rs = cpool.tile([128, KV], bf16, tag="rs", name="rs")
            nc.sync.dma_start(rs[:], rs_d[:])
            wk_t = load_rows(wkT_d, HID, "wk")
            mlo = cpool.tile([128, 128], bf16, tag="mlo", name="mlo")
            nc.sync.dma_start(mlo[:], mlo_d[:])
            mhi = cpool.tile([128, 128], bf16, tag="mhi", name="mhi")
            nc.sync.dma_start(mhi[:], mhi_d[:])
            wo_t = load_rows(woT_d, HID, "wo")

            qT = [cpool.tile([128, QR], bf16, tag=f"qT{h}", name=f"qT{h}")
                  for h in range(HP)]
            kT = [cpool.tile([128, KV], bf16, tag=f"kT{h}", name=f"kT{h}")
                  for h in range(HP)]
            vv = [cpool.tile([128, 16, 65], bf16, tag=f"vv{b}", name=f"vv{b}")
                  for b in range(NB)]
            ctx = [cpool.tile([128, QR], bf16, tag=f"ctx{h}", name=f"ctx{h}")
                   for h in range(HP)]
            sumsA = cpool.tile([14, QR], f32, tag="sumsA", name="sumsA")
            sumsB = cpool.tile([2, QR], f32, tag="sumsB", name="sumsB")

            # ---- v projection (+ ones column) ----
            for rb in range(NB):
                nc.sync.dma_start(vv[rb][:, :, 64:65],
                                  vones_d[rb * 128:(rb + 1) * 128, :])
                for d2 in range(2):
                    v_ps = psA.tile([128, 8, 64], f32, tag="pj", name="vps")
                    for f in range(FC):
                        nc.tensor.matmul(
                            v_ps[:], xt[f][:, rb * 128:(rb + 1) * 128],
                            wv_t[f][:, d2 * 512:(d2 + 1) * 512],
                            start=(f == 0), stop=(f == FC - 1))
                    nc.vector.tensor_copy(vv[rb][:, d2 * 8:(d2 + 1) * 8, 0:64],
                                          v_ps[:])

            # ---- RoPE: dst[:, dc0:dc0+512] = rope(src_ps) ----
            def rope_apply(src_ps, dst, tc0, dc0):
                n = 512
                raw = pR.tile([128, n], bf16, tag="rraw", name="rraw")
                nc.vector.tensor_copy(raw[:], src_ps[:])
                swp = pR.tile([128, n], bf16, tag="rswp", name="rswp")
                for g in range(4):
                    pg = (g ^ 1) * 32
                    nc.sync.dma_start(swp[g * 32:(g + 1) * 32, :],
                                      raw[pg:pg + 32, :])
                nc.vector.tensor_mul(dst[:, dc0:dc0 + n], raw[:],
                                     rc[:, tc0:tc0 + n])
                t2 = pR.tile([128, n], bf16, tag="rt2", name="rt2")
                nc.vector.tensor_mul(t2[:], swp[:], rs[:, tc0:tc0 + n])
                nc.vector.tensor_add(dst[:, dc0:dc0 + n],
                                     dst[:, dc0:dc0 + n], t2[:])

            # ---- q^T / k^T projections with RoPE (as 3 pieces) ----
            def proj_pieces(hp):
                def q_piece():
                    q_ps = psA.tile([128, QR], f32, tag="pj", name="qps")
                    for f in range(FC):
                        nc.tensor.matmul(
                            q_ps[:], wq_t[f][:, hp * 128:(hp + 1) * 128],
                            xt[f][:, WINDOW:KV],
                            start=(f == 0), stop=(f == FC - 1))
                    rope_apply(q_ps, qT[hp], WINDOW, 0)

                def k_piece(rh):
                    def run():
                        k_ps = psA.tile([128, 512], f32, tag="pj", name="kps")
                        for f in range(FC):
                            nc.tensor.matmul(
                                k_ps[:], wk_t[f][:, hp * 128:(hp + 1) * 128],
                                xt[f][:, rh * 512:(rh + 1) * 512],
                                start=(f == 0), stop=(f == FC - 1))
                        rope_apply(k_ps, kT[hp], rh * 512, rh * 512)
                    return run

                return [q_piece, k_piece(0), k_piece(1)]

            # ---- attention for one head pair (h0/h1 share st/p tiles) ----
            B_ORDER = [4, 5, 6, 7, 0, 1, 2, 3]  # b=4 first: full-width write
            LAG = 2

            def attn_pieces(hp):
                state = {}
                pbuf = {}

                def stage_st(b):
                    tlo, thi = max(0, b - 4), min(QT - 1, b)
                    ncols = (thi - tlo + 1) * 128
                    p = pP.tile([128, 2, 512], bf16, tag="p", name="p")
                    sts = []
                    for h01 in range(2):
                        po = h01 * 64
                        st = psS.tile([128, 512], f32, tag="st", name="st")
                        nc.tensor.matmul(
                            st[:, :ncols],
                            kT[hp][po:po + 64, b * 128:(b + 1) * 128],
                            qT[hp][po:po + 64, tlo * 128:(thi + 1) * 128],
                            start=True, stop=True, tile_position=(po, 0))
                        sts.append(st)
                    for h01 in range(2):
                        nc.scalar.activation(p[:, h01, :ncols],
                                             sts[h01][:, :ncols], Exp)
                    if b <= QT - 1:
                        c0 = (b - tlo) * 128
                        for h01 in range(2):
                            nc.vector.tensor_mul(p[:, h01, c0:c0 + 128],
                                                 p[:, h01, c0:c0 + 128], mlo[:])
                    if b >= 4:
                        for h01 in range(2):
                            nc.vector.tensor_mul(p[:, h01, 0:128],
                                                 p[:, h01, 0:128], mhi[:])
                    pbuf[b] = p

                def stage_pv(b):
                    tlo, thi = max(0, b - 4), min(QT - 1, b)
                    ncols = (thi - tlo + 1) * 128
                    p = pbuf.pop(b)
                    for h01 in range(2):
                        h = 2 * hp + h01
                        nc.tensor.matmul(
                            state["ctx_ps"][h01][:, tlo * 128:(thi + 1) * 128],
                            vv[b][:, h:h + 1, :], p[:, h01, :ncols],
                            start=(b == 4), stop=(b == B_ORDER[-1]),
                            skip_group_check=True)

                def alloc_piece():
                    state["ctx_ps"] = [
                        psC.tile([65, QR], f32, tag="ctx", name="ctxps")
                        for _ in range(2)]

                def fin_piece():
                    for h01 in range(2):
                        h = 2 * hp + h01
                        po = h01 * 64
                        stg = pR.tile([1, QR], f32, tag="sstg", name="sstg")
                        nc.scalar.copy(stg[:], state["ctx_ps"][h01][64:65, :])
                        if h < 14:
                            nc.sync.dma_start(sumsA[h:h + 1, :], stg[:])
                        else:
                            nc.sync.dma_start(sumsB[h - 14:h - 13, :], stg[:])
                        nc.vector.tensor_copy(ctx[hp][po:po + 64, :],
                                              state["ctx_ps"][h01][0:64, :])

                pieces = [alloc_piece]
                def st_piece(b):
                    return lambda: stage_st(b)
                def pv_piece(b):
                    return lambda: stage_pv(b)
                for i, b in enumerate(B_ORDER):
                    pieces.append(st_piece(b))
                    if i >= LAG:
                        pieces.append(pv_piece(B_ORDER[i - LAG]))
                for b in B_ORDER[-LAG:]:
                    pieces.append(pv_piece(b))
                pieces.append(fin_piece)
                return pieces

            def normalize(hps, recb, h0):
                for hp in hps:
                    bc = pW.tile([128, QR], bf16, tag="bc", name="bc", bufs=2)
                    for h01 in range(2):
                        h, po = 2 * hp + h01, h01 * 64
                        rb0 = pR.tile([1, QR], bf16, tag="rb0", name="rb0")
                        nc.sync.dma_start(rb0[:], recb[h - h0:h - h0 + 1, :])
                        bch = pR.tile([64, QR], bf16, tag="bch", name="bch")
                        nc.gpsimd.partition_broadcast(bch[:], rb0[:])
                        nc.sync.dma_start(bc[po:po + 64, :], bch[:])
                    nc.vector.tensor_mul(ctx[hp][:], ctx[hp][:], bc[:])

            def interleave(ap, pp):
                # spread proj pieces into the attn piece stream
                out_, pi = [], 0
                for i, a in enumerate(ap):
                    out_.append(a)
                    if pi < len(pp) and i in (1, 4, 7):
                        out_.append(pp[pi]); pi += 1
                out_.extend(pp[pi:])
                return out_

            for fn in proj_pieces(0):
                fn()
            for hp in range(1, HP):
                for fn in interleave(attn_pieces(hp - 1), proj_pieces(hp)):
                    fn()
            # normalize heads 0..13 while attn(7) runs
            recbA = pW.tile([14, QR], bf16, tag="recbA", name="recbA")
            with nc.allow_low_precision(reason="softmax denom fits bf16"):
                nc.vector.reciprocal(recbA[:], sumsA[:])
            normalize(range(HP - 1), recbA, 0)

            def o_partA(ti, n2):
                o_ps = psA.tile([128, 512], f32, tag="pj", name="ops")
                for f in range(FC - 1):
                    nc.tensor.matmul(
                        o_ps[:], ctx[f][:, ti * 128:(ti + 1) * 128],
                        wo_t[f][:, n2 * 512:(n2 + 1) * 512],
                        start=(f == 0), stop=(f == FC - 2))
                return o_ps

            def o_partB(ti, n2, o_ps, ob):
                f = FC - 1
                nc.tensor.matmul(
                    o_ps[:], ctx[f][:, ti * 128:(ti + 1) * 128],
                    wo_t[f][:, n2 * 512:(n2 + 1) * 512],
                    start=False, stop=True, skip_group_check=True)
                nc.any.tensor_copy(ob[:, n2 * 512:(n2 + 1) * 512], o_ps[:])

            ap7 = attn_pieces(HP - 1)
            held = {}
            for i, fn in enumerate(ap7):
                fn()
                if i == 3:
                    held[(0, 0)] = o_partA(0, 0)
                elif i == 6:
                    held[(0, 1)] = o_partA(0, 1)
            recbB = pW.tile([2, QR], bf16, tag="recbB", name="recbB")
            with nc.allow_low_precision(reason="softmax denom fits bf16"):
                nc.vector.reciprocal(recbB[:], sumsB[:])
            normalize([HP - 1], recbB, 14)

            # ---- output projection ----
            for ti in range(QT):
                ob = pW.tile([128, HID], f32, tag="ob", name="ob", bufs=3)
                for n2 in range(2):
                    if (ti, n2) in held:
                        o_partB(ti, n2, held.pop((ti, n2)), ob)
                    else:
                        o_ps = o_partA(ti, n2)
                        o_partB(ti, n2, o_ps, ob)
                nc.sync.dma_start(out_d[ti * 128:(ti + 1) * 128, :], ob[:])

    nc.compile()
    return nc


def _host_prep(x, wq, wk, wv, wo):
    bf = ml_dtypes.bfloat16
    xT = np.ascontiguousarray(x.T).astype(np.float32)  # [HID, T]
    wqT = np.ascontiguousarray((wq.astype(np.float32) * 0.125).T).astype(bf)
    wkT = np.ascontiguousarray(wk.T).astype(bf)
    wvT = np.ascontiguousarray(wv.T).astype(bf)
    woT = np.ascontiguousarray(wo.T).astype(bf)
    mlo = np.greater_equal.outer(np.arange(128), np.arange(128)).astype(bf)
    mhi = np.less_equal.outer(np.arange(128), np.arange(128)).astype(bf)

    inv_freq = ROPE_THETA ** (-np.arange(0, HD, 2, dtype=np.float64) / HD)  # [32]
    d_idx = np.arange(128) % HD
    freq_i = d_idx % 32
    sign = np.where(d_idx < 32, -1.0, 1.0)

    in_maps = []
    for c in range(NCORES):
        lo = c * QR - WINDOW
        xkv = np.zeros((HID, KV), np.float32)
        if lo < 0:
            xkv[:, -lo:] = xT[:, 0:lo + KV]
        else:
            xkv[:] = xT[:, lo:lo + KV]
        vones = np.ones((KV, 16), np.float32)
        if lo < 0:
            vones[0:-lo, :] = 0.0
        pos = lo + np.arange(KV, dtype=np.float64)  # [KV]
        ang = pos[None, :] * inv_freq[freq_i][:, None]  # [128, KV]
        rcos = np.cos(ang).astype(bf)
        rsin = (sign[:, None] * np.sin(ang)).astype(bf)
        in_maps.append({
            "xT": xkv.astype(bf),
            "wqT": wqT, "wkT": wkT, "wvT": wvT, "woT": woT,
            "vones": vones.astype(bf),
            "mlo": mlo, "mhi": mhi,
            "ropecos": rcos, "ropesin": rsin,
        })
    return in_maps


def _run(x, wq, wk, wv, wo, trace=False, tmpdir=None):
    from concourse.bass_utils import run_bass_kernel_spmd
    if "nc" not in _CACHE:
        _CACHE["nc"] = _build_program()
    nc = _CACHE["nc"]
    in_maps = _host_prep(x, wq, wk, wv, wo)
    res = run_bass_kernel_spmd(nc, in_maps, list(range(NCORES)),
                               trace=trace, tmpdir=tmpdir)
    out = np.concatenate([res.results[c]["out"] for c in range(NCORES)], axis=0)
    return np.ascontiguousarray(out).astype(np.float32), res


def kernel(x, wq, wk, wv, wo):
    out, _ = _run(x, wq, wk, wv, wo)
    return out


# revision 21
# speedup vs baseline: 1.5738x; 1.0077x over previous
"""Trainium2 Bass kernel for nn_MimiAttention (sliding-window causal attention).

Reference math (T=4096, HID=1024, 16 heads x 64 dims, window 512, RoPE):
  q = rope(x @ wq.T); k = rope(x @ wk.T); v = x @ wv.T
  ctx = sdpa(q, k, v, causal, local_window=(512, 0), scale=1/8)
  out = ctx @ wo.T

Sharding: sequence-parallel across 8 NeuronCores, zero communication.
Core c owns queries [c*512, (c+1)*512) and recomputes k/v over its kv
window [c*512-512, (c+1)*512) (halo recompute).

On-device layout: everything transposed (feature dim on partitions).
Softmax without max-subtraction (scores are small: |S/8| < ~4), row sums
via a ones-column appended to V, triangle masks as bf16 multiplies on
exp(S^T), all 16 head denominators inverted in one batched reciprocal.
RoPE in the transposed layout: partner-dim swap via 4 small SBUF->SBUF
DMAs (partition shift), then two table multiplies + add.
"""

import sys

sys.path.insert(0, "/opt/trn_rl_repo")

import numpy as np
import ml_dtypes

T, HID, NH, HD = 4096, 1024, 16, 64
WINDOW = 512
ROPE_THETA = 10000.0
NCORES = 8
QR = T // NCORES          # 512 queries per core
KV = QR + WINDOW          # 1024 kv rows per core (incl. halo)
NB = KV // 128            # 8 kv blocks
QT = QR // 128            # 4 query tiles
HP = NH // 2              # 8 head pairs
FC = HID // 128           # 8 feature chunks

_CACHE = {}


def _build_program():
    import concourse.mybir as mybir
    import concourse.tile as tile
    from concourse import bacc

    f32 = mybir.dt.float32
    bf16 = mybir.dt.bfloat16
    Exp = mybir.ActivationFunctionType.Exp

    nc = bacc.Bacc("TRN2", target_bir_lowering=False, debug=False,
                   num_devices=NCORES)

    xT_d = nc.declare_dram_parameter("xT", [HID, KV], bf16, isOutput=False)
    wqT_d = nc.declare_dram_parameter("wqT", [HID, HID], bf16, isOutput=False)
    wkT_d = nc.declare_dram_parameter("wkT", [HID, HID], bf16, isOutput=False)
    wvT_d = nc.declare_dram_parameter("wvT", [HID, HID], bf16, isOutput=False)
    woT_d = nc.declare_dram_parameter("woT", [HID, HID], bf16, isOutput=False)
    vones_d = nc.declare_dram_parameter("vones", [KV, 16], bf16, isOutput=False)
    mlo_d = nc.declare_dram_parameter("mlo", [128, 128], bf16, isOutput=False)
    mhi_d = nc.declare_dram_parameter("mhi", [128, 128], bf16, isOutput=False)
    rc_d = nc.declare_dram_parameter("ropecos", [128, KV], bf16, isOutput=False)
    rs_d = nc.declare_dram_parameter("ropesin", [128, KV], bf16, isOutput=False)
    out_d = nc.declare_dram_parameter("out", [QR, HID], f32, isOutput=True)

    with tile.TileContext(nc) as tc:
        with (
            tc.tile_pool(name="const", bufs=1) as cpool,
            tc.tile_pool(name="psA", bufs=2, space="PSUM") as psA,
            tc.tile_pool(name="psS", bufs=4, space="PSUM") as psS,
            tc.tile_pool(name="psC", bufs=2, space="PSUM") as psC,
            tc.tile_pool(name="pP", bufs=6) as pP,
            tc.tile_pool(name="pR", bufs=3) as pR,
            tc.tile_pool(name="pW", bufs=2) as pW,
        ):
            # ---- constants / weights into SBUF (xt/wv first for v-proj) ----
            xt, wv_t = [], []
            for f in range(FC):
                t_ = cpool.tile([128, KV], bf16, tag=f"xt{f}", name=f"xt{f}")
                nc.sync.dma_start(t_[:], xT_d[f * 128:(f + 1) * 128, :])
                xt.append(t_)
                t_ = cpool.tile([128, HID], bf16, tag=f"wv{f}", name=f"wv{f}")
                nc.sync.dma_start(t_[:], wvT_d[f * 128:(f + 1) * 128, :])
                wv_t.append(t_)

            def load_rows(dram, n_free, tagp):
                ts_ = []
                for f in range(FC):
                    t_ = cpool.tile([128, n_free], bf16, tag=f"{tagp}{f}",
                                    name=f"{tagp}{f}")
                    nc.sync.dma_start(t_[:], dram[f * 128:(f + 1) * 128, :])
                    ts_.append(t_)
                return ts_

            wq_t = load_rows(wqT_d, HID, "wq")
            rc = cpool.tile([128, KV], bf16, tag="rc", name="rc")
            nc.sync.dma_start(rc[:], rc_d[:])
            rs = cpool.tile([128, KV], bf16, tag="rs", name="rs")
            nc.sync.dma_start(rs[:], rs_d[:])
            wk_t = load_rows(wkT_d, HID, "wk")
            mlo = cpool.tile([128, 128], bf16, tag="mlo", name="mlo")
            nc.sync.dma_start(mlo[:], mlo_d[:])
            mhi = cpool.tile([128, 128], bf16, tag="mhi", name="mhi")
            nc.sync.dma_start(mhi[:], mhi_d[:])
            wo_t = load_rows(woT_d, HID, "wo")

            qT = [cpool.tile([128, QR], bf16, tag=f"qT{h}", name=f"qT{h}")
                  for h in range(HP)]
            kT = [cpool.tile([128, KV], bf16, tag=f"kT{h}", name=f"kT{h}")
                  for h in range(HP)]
            vv = [cpool.tile([128, 16, 65], bf16, tag=f"vv{b}", name=f"vv{b}")
                  for b in range(NB)]
            ctx = [cpool.tile([128, QR], bf16, tag=f"ctx{h}", name=f"ctx{h}")
                   for h in range(HP)]
            sumsA = cpool.tile([14, QR], f32, tag="sumsA", name="sumsA")
            sumsB = cpool.tile([2, QR], f32, tag="sumsB", name="sumsB")

            # ---- v projection (+ ones column) ----
            for rb in range(NB):
                nc.sync.dma_start(vv[rb][:, :, 64:65],
                                  vones_d[rb * 128:(rb + 1) * 128, :])
                for d2 in range(2):
                    v_ps = psA.tile([128, 8, 64], f32, tag="pj", name="vps")
                    for f in range(FC):
                        nc.tensor.matmul(
                            v_ps[:], xt[f][:, rb * 128:(rb + 1) * 128],
                            wv_t[f][:, d2 * 512:(d2 + 1) * 512],
                            start=(f == 0), stop=(f == FC - 1))
                    nc.vector.tensor_copy(vv[rb][:, d2 * 8:(d2 + 1) * 8, 0:64],
                                          v_ps[:])

            # ---- RoPE: dst[:, dc0:dc0+512] = rope(src_ps) ----
            def rope_apply(src_ps, dst, tc0, dc0):
                n = 512
                raw = pR.tile([128, n], bf16, tag="rraw", name="rraw")
                nc.vector.tensor_copy(raw[:], src_ps[:])
                swp = pR.tile([128, n], bf16, tag="rswp", name="rswp")
                for g in range(4):
                    pg = (g ^ 1) * 32
                    nc.sync.dma_start(swp[g * 32:(g + 1) * 32, :],
                                      raw[pg:pg + 32, :])
                nc.vector.tensor_mul(dst[:, dc0:dc0 + n], raw[:],
                                     rc[:, tc0:tc0 + n])
                t2 = pR.tile([128, n], bf16, tag="rt2", name="rt2")
                nc.vector.tensor_mul(t2[:], swp[:], rs[:, tc0:tc0 + n])
                nc.vector.tensor_add(dst[:, dc0:dc0 + n],
                                     dst[:, dc0:dc0 + n], t2[:])

            # ---- q^T / k^T projections with RoPE (as 3 pieces) ----
            def proj_pieces(hp):
                def q_piece():
                    q_ps = psA.tile([128, QR], f32, tag="pj", name="qps")
                    for f in range(FC):
                        nc.tensor.matmul(
                            q_ps[:], wq_t[f][:, hp * 128:(hp + 1) * 128],
                            xt[f][:, WINDOW:KV],
                            start=(f == 0), stop=(f == FC - 1))
                    rope_apply(q_ps, qT[hp], WINDOW, 0)

                def k_piece(rh):
                    def run():
                        k_ps = psA.tile([128, 512], f32, tag="pj", name="kps")
                        for f in range(FC):
                            nc.tensor.matmul(
                                k_ps[:], wk_t[f][:, hp * 128:(hp + 1) * 128],
                                xt[f][:, rh * 512:(rh + 1) * 512],
                                start=(f == 0), stop=(f == FC - 1))
                        rope_apply(k_ps, kT[hp], rh * 512, rh * 512)
                    return run

                return [q_piece, k_piece(0), k_piece(1)]

            # ---- attention for one head pair (h0/h1 share st/p tiles) ----
            B_ORDER = [4, 5, 6, 7, 0, 1, 2, 3]  # b=4 first: full-width write
            LAG = 4

            def attn_pieces(hp):
                state = {}
                pbuf = {}

                def stage_st(b):
                    tlo, thi = max(0, b - 4), min(QT - 1, b)
                    ncols = (thi - tlo + 1) * 128
                    p = pP.tile([128, 2, 512], bf16, tag="p", name="p")
                    sts = []
                    for h01 in range(2):
                        po = h01 * 64
                        st = psS.tile([128, 512], f32, tag="st", name="st")
                        nc.tensor.matmul(
                            st[:, :ncols],
                            kT[hp][po:po + 64, b * 128:(b + 1) * 128],
                            qT[hp][po:po + 64, tlo * 128:(thi + 1) * 128],
                            start=True, stop=True, tile_position=(po, 0))
                        sts.append(st)
                    for h01 in range(2):
                        nc.scalar.activation(p[:, h01, :ncols],
                                             sts[h01][:, :ncols], Exp)
                    if b <= QT - 1:
                        c0 = (b - tlo) * 128
                        for h01 in range(2):
                            nc.vector.tensor_mul(p[:, h01, c0:c0 + 128],
                                                 p[:, h01, c0:c0 + 128], mlo[:])
                    if b >= 4:
                        for h01 in range(2):
                            nc.vector.tensor_mul(p[:, h01, 0:128],
                                                 p[:, h01, 0:128], mhi[:])
                    pbuf[b] = p

                def stage_pv(b):
                    tlo, thi = max(0, b - 4), min(QT - 1, b)
                    ncols = (thi - tlo + 1) * 128
                    p = pbuf.pop(b)
                    for h01 in range(2):
                        h = 2 * hp + h01
                        nc.tensor.matmul(
                            state["ctx_ps"][h01][:, tlo * 128:(thi + 1) * 128],
                            vv[b][:, h:h + 1, :], p[:, h01, :ncols],
                            start=(b == 4), stop=(b == B_ORDER[-1]),
                            skip_group_check=True)

                def alloc_piece():
                    state["ctx_ps"] = [
                        psC.tile([65, QR], f32, tag="ctx", name="ctxps")
                        for _ in range(2)]

                def fin_piece():
                    for h01 in range(2):
                        h = 2 * hp + h01
                        po = h01 * 64
                        stg = pR.tile([1, QR], f32, tag="sstg", name="sstg")
                        nc.scalar.copy(stg[:], state["ctx_ps"][h01][64:65, :])
                        if h < 14:
                            nc.sync.dma_start(sumsA[h:h + 1, :], stg[:])
                        else:
                            nc.sync.dma_start(sumsB[h - 14:h - 13, :], stg[:])
                        nc.vector.tensor_copy(ctx[hp][po:po + 64, :],
                                              state["ctx_ps"][h01][0:64, :])

                pieces = [alloc_piece]
                def st_piece(b):
                    return lambda: stage_st(b)
                def pv_piece(b):
                    return lambda: stage_pv(b)
                for i, b in enumerate(B_ORDER):
                    pieces.append(st_piece(b))
                    if i >= LAG:
                        pieces.append(pv_piece(B_ORDER[i - LAG]))
                for b in B_ORDER[-LAG:]:
                    pieces.append(pv_piece(b))
                pieces.append(fin_piece)
                return pieces

            def normalize(hps, recb, h0):
                for hp in hps:
                    bc = pW.tile([128, QR], bf16, tag="bc", name="bc", bufs=2)
                    for h01 in range(2):
                        h, po = 2 * hp + h01, h01 * 64
                        rb0 = pR.tile([1, QR], bf16, tag="rb0", name="rb0")
                        nc.sync.dma_start(rb0[:], recb[h - h0:h - h0 + 1, :])
                        bch = pR.tile([64, QR], bf16, tag="bch", name="bch")
                        nc.gpsimd.partition_broadcast(bch[:], rb0[:])
                        nc.sync.dma_start(bc[po:po + 64, :], bch[:])
                    nc.vector.tensor_mul(ctx[hp][:], ctx[hp][:], bc[:])

            def interleave(ap, pp):
                # spread proj pieces into the attn piece stream
                out_, pi = [], 0
                for i, a in enumerate(ap):
                    out_.append(a)
                    if pi < len(pp) and i in (1, 4, 7):
                        out_.append(pp[pi]); pi += 1
                out_.extend(pp[pi:])
                return out_

            for fn in proj_pieces(0):
                fn()
            for hp in range(1, HP):
                for fn in interleave(attn_pieces(hp - 1), proj_pieces(hp)):
                    fn()
            # normalize heads 0..13 while attn(7) runs
            recbA = pW.tile([14, QR], bf16, tag="recbA", name="recbA")
            with nc.allow_low_precision(reason="softmax denom fits bf16"):
                nc.vector.reciprocal(recbA[:], sumsA[:])
            normalize(range(HP - 1), recbA, 0)

            def o_partA(ti, n2):
                o_ps = psA.tile([128, 512], f32, tag="pj", name="ops")
                for f in range(FC - 1):
                    nc.tensor.matmul(
                        o_ps[:], ctx[f][:, ti * 128:(ti + 1) * 128],
                        wo_t[f][:, n2 * 512:(n2 + 1) * 512],
                        start=(f == 0), stop=(f == FC - 2))
                return o_ps

            def o_partB(ti, n2, o_ps, ob):
                f = FC - 1
                nc.tensor.matmul(
                    o_ps[:], ctx[f][:, ti * 128:(ti + 1) * 128],
                    wo_t[f][:, n2 * 512:(n2 + 1) * 512],
                    start=False, stop=True, skip_group_check=True)
                nc.any.tensor_copy(ob[:, n2 * 512:(n2 + 1) * 512], o_ps[:])

            ap7 = attn_pieces(HP - 1)
            held = {}
            for i, fn in enumerate(ap7):
                fn()
                if i == 3:
                    held[(0, 0)] = o_partA(0, 0)
                elif i == 6:
                    held[(0, 1)] = o_partA(0, 1)
            recbB = pW.tile([2, QR], bf16, tag="recbB", name="recbB")
            with nc.allow_low_precision(reason="softmax denom fits bf16"):
                nc.vector.reciprocal(recbB[:], sumsB[:])
            normalize([HP - 1], recbB, 14)

            # ---- output projection ----
            for ti in range(QT):
                ob = pW.tile([128, HID], f32, tag="ob", name="ob", bufs=3)
                for n2 in range(2):
                    if (ti, n2) in held:
                        o_partB(ti, n2, held.pop((ti, n2)), ob)
                    else:
                        o_ps = o_partA(ti, n2)
                        o_partB(ti, n2, o_ps, ob)
                nc.sync.dma_start(out_d[ti * 128:(ti + 1) * 128, :], ob[:])

    nc.compile()
    return nc


def _host_prep(x, wq, wk, wv, wo):
    bf = ml_dtypes.bfloat16
    xT = np.ascontiguousarray(x.T).astype(np.float32)  # [HID, T]
    wqT = np.ascontiguousarray((wq.astype(np.float32) * 0.125).T).astype(bf)
    wkT = np.ascontiguousarray(wk.T).astype(bf)
    wvT = np.ascontiguousarray(wv.T).astype(bf)
    woT = np.ascontiguousarray(wo.T).astype(bf)
    mlo = np.greater_equal.outer(np.arange(128), np.arange(128)).astype(bf)
    mhi = np.less_equal.outer(np.arange(128), np.arange(128)).astype(bf)

    inv_freq = ROPE_THETA ** (-np.arange(0, HD, 2, dtype=np.float64) / HD)  # [32]
    d_idx = np.arange(128) % HD
    freq_i = d_idx % 32
    sign = np.where(d_idx < 32, -1.0, 1.0)

    in_maps = []
    for c in range(NCORES):
        lo = c * QR - WINDOW
        xkv = np.zeros((HID, KV), np.float32)
        if lo < 0:
            xkv[:, -lo:] = xT[:, 0:lo + KV]
        else:
            xkv[:] = xT[:, lo:lo + KV]
        vones = np.ones((KV, 16), np.float32)
        if lo < 0:
            vones[0:-lo, :] = 0.0
        pos = lo + np.arange(KV, dtype=np.float64)  # [KV]
        ang = pos[None, :] * inv_freq[freq_i][:, None]  # [128, KV]
        rcos = np.cos(ang).astype(bf)
        rsin = (sign[:, None] * np.sin(ang)).astype(bf)
        in_maps.append({
            "xT": xkv.astype(bf),
            "wqT": wqT, "wkT": wkT, "wvT": wvT, "woT": woT,
            "vones": vones.astype(bf),
            "mlo": mlo, "mhi": mhi,
            "ropecos": rcos, "ropesin": rsin,
        })
    return in_maps


def _run(x, wq, wk, wv, wo, trace=False, tmpdir=None):
    from concourse.bass_utils import run_bass_kernel_spmd
    if "nc" not in _CACHE:
        _CACHE["nc"] = _build_program()
    nc = _CACHE["nc"]
    in_maps = _host_prep(x, wq, wk, wv, wo)
    res = run_bass_kernel_spmd(nc, in_maps, list(range(NCORES)),
                               trace=trace, tmpdir=tmpdir)
    out = np.concatenate([res.results[c]["out"] for c in range(NCORES)], axis=0)
    return np.ascontiguousarray(out).astype(np.float32), res


def kernel(x, wq, wk, wv, wo):
    out, _ = _run(x, wq, wk, wv, wo)
    return out
